# revision 1
# baseline (speedup 1.0000x reference)
"""GAT (2-layer, 8-head) Trainium2 kernel, 8-core SPMD.

Phase 1 (head-parallel): core h computes head h over the full graph.
  Identity used: with s = f1_i + f2_j,
    p = exp(leakyrelu(s)) * mask
      = u2[j] * max(u1[i], v1[i] * w2[j]) * m[j,i]
  where u1 = e^{f1}, v1 = e^{0.2 f1}, u2 = e^{f2}, w2 = e^{-0.8 f2}.
  The u2[j] factor is folded into the matmul lhsT (Wh * u2 per j-chunk),
  so the [N,N] score needs only 3 elementwise passes, split across
  Act / DVE / Pool engines (bf16 mask keeps the DVE 2x tensor mode).
  Phase 1 also computes this head's partial of the layer-2 projection
  Wh2_h = elu(head_out) @ W_out[64h:64h+64] on-device, so the head
  features never leave the device.
Phase 2 (row-parallel): host sums the 8 partial Wh2 (+ f1o/f2o columns),
  precomputes the exp vectors; each core runs layer-2 attention for its
  512 rows in [i, f] orientation (no transposes in the epilogue) and
  finishes with elu + log_softmax.
"""

import sys

for p in ("/opt/trn_rl_repo", "/opt/pypackages"):
    if p not in sys.path:
        sys.path.append(p)

import numpy as np
import ml_dtypes

import concourse.bass as bass
import concourse.bacc as bacc
import concourse.tile as tile
from concourse import mybir
from concourse.bass_utils import run_bass_kernel_spmd
from concourse.masks import make_identity

BF16 = mybir.dt.bfloat16
FP8 = mybir.dt.float8e4
F32 = mybir.dt.float32
AX = mybir.AxisListType
OP = mybir.AluOpType
AF = mybir.ActivationFunctionType

N, FIN, HID, HEADS, FOUT = 4096, 512, 64, 8, 256
NCORES = 8
ALPHA = 0.2

# engine split points for phase-1 attention elementwise work
C1 = 4096   # t1: Act does cols [0, C1), DVE does [C1, N)
C3L = 2368  # t3: DVE does cols [0, C3L), Pool does [C3L, N)


def build_phase1(n=N, fin=FIN, hid=HID, fout=FOUT):
    """Per-core: xT [fin, n] bf16, maskT [n, n] bf16, wcat [fin, hid] bf16,
    wocat [hid, fout+2] bf16 (W_out rows for this head | woa1 | woa2),
    u1v1 [2, n] bf16 (exp(f1), exp(0.2 f1)),
    u2w2 [128, n/128, 2] f32 (exp(f2), exp(-0.8 f2), partition-major)
    -> wh2p [n, fout+2] bf16 (partial Wh2 | f1o | f2o partials)."""
    nc = bacc.Bacc("TRN2", target_bir_lowering=False, debug=False,
                   enable_asserts=False)
    kch = fin // 128          # contraction chunks for x@W
    nch = n // 128            # 128-row chunks of nodes
    nib = n // 512            # 512-col i-blocks
    woc = fout + 2

    xT = nc.dram_tensor("xT", [fin, n], BF16, kind="ExternalInput")
    maskT = nc.dram_tensor("maskT", [n, n], BF16, kind="ExternalInput")
    wcat = nc.dram_tensor("wcat", [fin, hid], BF16, kind="ExternalInput")
    wocat = nc.dram_tensor("wocat", [hid + 1, woc], BF16,
                           kind="ExternalInput")
    u1v1 = nc.dram_tensor("u1v1", [2, n], BF16, kind="ExternalInput")
    u2w2 = nc.dram_tensor("u2w2", [128, nch, 2], F32, kind="ExternalInput")
    wh2p = nc.dram_tensor("wh2p", [n, woc], BF16, kind="ExternalOutput")
    scrR = nc.dram_tensor("scrR", [n], BF16)

    with tile.TileContext(nc) as tc:
        with tc.tile_pool(name="consts", bufs=1) as consts:
            id128 = consts.tile([128, 128], F32)
            make_identity(nc, id128[:])
            warm = consts.tile([1, 1], F32)
            nc.scalar.activation(out=warm[:], in_=id128[0:1, 0:1],
                                 func=AF.Copy)
            id64b = consts.tile([hid, hid], BF16)
            nc.vector.tensor_copy(out=id64b[:], in_=id128[0:hid, 0:hid])
            wsb = consts.tile([128, kch, hid], BF16)
            nc.sync.dma_start(
                out=wsb[:],
                in_=bass.AP(tensor=wcat, offset=0,
                            ap=[[hid, 128], [128 * hid, kch], [1, hid]]))
            wosb = consts.tile([hid + 1, woc], BF16)
            nc.sync.dma_start(out=wosb[:], in_=wocat[:, :])
            uwsb = consts.tile([128, nch, 2], F32)
            nc.sync.dma_start(out=uwsb[:], in_=u2w2[:, :, :])
            u1b = consts.tile([128, n], BF16)
            v1b = consts.tile([128, n], BF16)
            nc.sync.dma_start(
                out=v1b[:],
                in_=bass.AP(tensor=u1v1, offset=n, ap=[[0, 128], [1, n]]))
            nc.sync.dma_start(
                out=u1b[:],
                in_=bass.AP(tensor=u1v1, offset=0, ap=[[0, 128], [1, n]]))

            whR = consts.tile([128, nch, hid + 1], BF16)
            nc.vector.memset(whR[:, :, hid:hid + 1], 1.0)

            # ---- WhT = W^T @ x^T by 512-col blocks, transposed back, ----
            # ---- overlapped with the attention elementwise pipeline   ----
            # Attention SBUF pools are opened alongside the x pool so the
            # t1/t2/t3 pipeline starts while Wh is still being produced;
            # only the accumulating attention matmuls wait (their PSUM pool
            # opens after the Wh pipeline's PSUM closes).  Copies off the
            # Wh path run on Act/Pool to keep DVE free for the loop.
            with (
                tc.tile_pool(name="mpool", bufs=4) as mpool,
                tc.tile_pool(name="t1pool", bufs=3) as t1pool,
                tc.tile_pool(name="t2pool", bufs=3) as t2pool,
                tc.tile_pool(name="t3pool", bufs=6) as t3pool,
                tc.tile_pool(name="whup", bufs=2) as whup,
            ):
                mts = {}
                with tc.tile_pool(name="xpool", bufs=1) as xpool:
                    xsb = xpool.tile([128, kch, n], BF16)
                    with tc.tile_pool(name="psW", bufs=4,
                                      space="PSUM") as psW:
                        for half in range(2):
                            hsl = slice(half * (n // 2),
                                        (half + 1) * (n // 2))
                            for kc in range(kch):
                                nc.sync.dma_start(
                                    out=xsb[:, kc, hsl],
                                    in_=xT[kc * 128:(kc + 1) * 128, hsl])
                            if half == 0:
                                # prefetch first masks between the x halves
                                for jp in range(2):
                                    mt = mpool.tile([128, n], BF16,
                                                    name=f"mtp{jp}",
                                                    tag="mt")
                                    mts[jp] = mt
                                    nc.sync.dma_start(
                                        out=mt[:],
                                        in_=maskT[jp * 128:(jp + 1) * 128,
                                                  :])
                            # node-major Wh production: no transposes, no
                            # wtmp copies — keeps DVE (the pacer) free
                            for i16 in range(nch // 2):
                                i = half * (nch // 2) + i16
                                pw = psW.tile([128, hid], F32)
                                for kc in range(kch):
                                    nc.tensor.matmul(
                                        out=pw[:],
                                        lhsT=xsb[:, kc,
                                                 i * 128:(i + 1) * 128],
                                        rhs=wsb[:, kc, :],
                                        start=(kc == 0),
                                        stop=(kc == kch - 1))
                                nc.vector.tensor_copy(
                                    out=whR[:, i, 0:hid], in_=pw[:])

                # ---- attention: p = u2[j]*max(u1[i], v1[i]*w2[j])*m ----
                with tc.tile_pool(name="atps", bufs=nib,
                                  space="PSUM") as atps:
                    pss = [atps.tile([hid + 1, 512], F32, name=f"pss{_i}",
                                     tag="pss") for _i in range(nib)]
                    for jc in range(nch):
                        if jc in mts:
                            mt = mts[jc]
                        else:
                            mt = mpool.tile([128, n], BF16, tag="mt")
                            nc.sync.dma_start(
                                out=mt[:],
                                in_=maskT[jc * 128:(jc + 1) * 128, :])
                        whu = whup.tile([128, hid + 1], BF16)
                        nc.vector.tensor_scalar_mul(whu[:], whR[:, jc, :],
                                                    uwsb[:, jc, 0:1])
                        t1 = t1pool.tile([128, n], BF16)
                        nc.scalar.activation(out=t1[:, 0:C1],
                                             in_=v1b[:, 0:C1],
                                             func=AF.Copy,
                                             scale=uwsb[:, jc, 1:2])
                        if C1 < n:
                            nc.vector.tensor_scalar_mul(t1[:, C1:n],
                                                        v1b[:, C1:n],
                                                        uwsb[:, jc, 1:2])
                        t2 = t2pool.tile([128, n], BF16)
                        nc.vector.tensor_max(t2[:], t1[:], u1b[:])
                        t3 = t3pool.tile([128, n], BF16)
                        nc.vector.tensor_mul(t3[:, 0:C3L], t2[:, 0:C3L],
                                             mt[:, 0:C3L])
                        nc.gpsimd.tensor_mul(t3[:, C3L:n], t2[:, C3L:n],
                                             mt[:, C3L:n])
                        for ib in range(nib):
                            nc.tensor.matmul(
                                out=pss[ib][:],
                                lhsT=whu[:],
                                rhs=t3[:, ib * 512:(ib + 1) * 512],
                                start=(jc == 0), stop=(jc == nch - 1))

                    # ---- epilogue: normalize + elu + partial Wh2 ----
                    # per-quarter recip bounce, eighth-granularity pipeline;
                    # elu(h) = max(h, exp(min(h,0)) - 1) needs one op fewer;
                    # psum slots for the Wh2 matmuls recycle the attention
                    # banks; per-eighth batched output DMA.
                    with tc.tile_pool(name="ep1", bufs=2) as ep1:
                        rcps = [consts.tile([1, 512], BF16,
                                            name=f"rcp{_i}")
                                for _i in range(nib)]
                        rsbs = [consts.tile([hid, 512], BF16,
                                            name=f"rsb{_i}")
                                for _i in range(nib)]
                        for ib in range(nib):
                            with nc.allow_low_precision(
                                    reason="bf16 recip for DRAM bounce"):
                                nc.vector.reciprocal(
                                    out=rcps[ib][:],
                                    in_=pss[ib][hid:hid + 1, :])
                            nc.sync.dma_start(
                                out=bass.AP(tensor=scrR, offset=ib * 512,
                                            ap=[[n, 1], [1, 512]]),
                                in_=rcps[ib][:])
                            nc.sync.dma_start(
                                out=rsbs[ib][:],
                                in_=bass.AP(tensor=scrR, offset=ib * 512,
                                            ap=[[0, hid], [1, 512]]))
                        hCs = [consts.tile([hid + 1, 512], BF16,
                                           name=f"hC{_i}")
                               for _i in range(2)]
                        for ib in range(nib):
                            hv = ep1.tile([hid, 512], BF16, name="hv",
                                          tag="hv")
                            nc.vector.tensor_mul(hv[:], pss[ib][0:hid, :],
                                                 rsbs[ib][:])
                            em = ep1.tile([hid, 512], BF16, name="em",
                                          tag="em")
                            nc.scalar.activation(out=em[:], in_=hv[:],
                                                 func=AF.Exp)
                            # elu = max(hv, min(exp(hv),1) - 1): exp(hv)
                            # needs no pre-clamp (hv <= ~6, no overflow),
                            # the clamp+shift is one 2-scalar-op TSP, and
                            # the final max runs on the idle Pool engine
                            emm = ep1.tile([hid, 512], BF16, name="emm",
                                           tag="emm")
                            nc.vector.tensor_scalar(out=emm[:], in0=em[:],
                                                    scalar1=1.0,
                                                    scalar2=-1.0,
                                                    op0=OP.min, op1=OP.add)
                            hC = hCs[ib % 2]
                            nc.vector.tensor_max(hC[0:hid, :], emm[:],
                                                 hv[:])
                            wout = ep1.tile([128, 4, woc], BF16, name="wout",
                                            tag="wout", bufs=3)
                            for t4 in range(4):
                                wp = atps.tile([128, woc], F32, name="wp",
                                               tag="pss")
                                nc.tensor.matmul(
                                    out=wp[:],
                                    lhsT=hC[0:hid, t4 * 128:(t4 + 1) * 128],
                                    rhs=wosb[0:hid, :],
                                    start=True, stop=True)
                                if (ib * 4 + t4) % 4 != 1:
                                    nc.scalar.activation(out=wout[:, t4, :],
                                                         in_=wp[:],
                                                         func=AF.Copy)
                                else:
                                    nc.vector.tensor_copy(out=wout[:, t4, :],
                                                          in_=wp[:])
                            nc.sync.dma_start(
                                out=bass.AP(tensor=wh2p,
                                            offset=ib * 512 * woc,
                                            ap=[[woc, 128], [128 * woc, 4],
                                                [1, woc]]),
                                in_=wout[:])

    nc.compile()
    return nc


def build_phase2(n=N, fout=FOUT):
    """Per-core: wh2 [n, fout+1] bf16 (Wh2 | ones), m2 [n, rows] bf16,
    u1v2 [2, rows] bf16 (exp(f1o) | exp(0.2 f1o) for own rows),
    u2w2 [128, nch, 2] f32 (exp(f2o), exp(-0.8 f2o), partition-major)
    -> out [rows, fout] f32 (log_softmax rows)."""
    nc = bacc.Bacc("TRN2", target_bir_lowering=False, debug=False,
                   enable_asserts=False)
    rows = n // NCORES
    nch = n // 128
    rch = rows // 128
    wc = fout + 1

    wh2 = nc.dram_tensor("wh2", [n, wc], BF16, kind="ExternalInput")
    m2 = nc.dram_tensor("m2", [n, rows], BF16, kind="ExternalInput")
    u1v2 = nc.dram_tensor("u1v2", [2, rows], BF16, kind="ExternalInput")
    u2w2 = nc.dram_tensor("u2w2", [128, nch, 2], F32, kind="ExternalInput")
    out = nc.dram_tensor("out", [rows, fout], F32, kind="ExternalOutput")

    with tile.TileContext(nc) as tc:
        with tc.tile_pool(name="consts", bufs=1) as consts:
            u2w2sb = consts.tile([128, nch, 2], F32)
            nc.sync.dma_start(out=u2w2sb[:], in_=u2w2[:, :, :])
            u1b = consts.tile([128, rows], BF16)
            v1b = consts.tile([128, rows], BF16)
            nc.sync.dma_start(
                out=u1b[:],
                in_=bass.AP(tensor=u1v2, offset=0, ap=[[0, 128], [1, rows]]))
            nc.sync.dma_start(
                out=v1b[:],
                in_=bass.AP(tensor=u1v2, offset=rows,
                            ap=[[0, 128], [1, rows]]))
            whsbs = [consts.tile([128, 4, wc], BF16, name=f"whsb{_g}")
                     for _g in range(nch // 4)]
            mts = [consts.tile([128, 4, rows], BF16, name=f"mt{_g}")
                   for _g in range(nch // 4)]

            with (
                tc.tile_pool(name="t1pool", bufs=3) as t1pool,
                tc.tile_pool(name="t2pool", bufs=3) as t2pool,
                tc.tile_pool(name="t3pool", bufs=3) as t3pool,
                tc.tile_pool(name="whup", bufs=2) as whup,
                tc.tile_pool(name="ps4", bufs=rch, space="PSUM") as ps4,
            ):
                po = [ps4.tile([128, wc], F32, name=f"po{_i}", tag="po")
                      for _i in range(rch)]
                for g in range(nch // 4):
                    nc.sync.dma_start(
                        out=whsbs[g][:],
                        in_=bass.AP(tensor=wh2, offset=g * 512 * wc,
                                    ap=[[wc, 128], [128 * wc, 4], [1, wc]]))
                    nc.sync.dma_start(
                        out=mts[g][:],
                        in_=bass.AP(tensor=m2, offset=g * 512 * rows,
                                    ap=[[rows, 128], [128 * rows, 4],
                                        [1, rows]]))
                for jc in range(nch):
                    g, j4 = jc // 4, jc % 4
                    whu = whup.tile([128, wc], BF16)
                    nc.vector.tensor_scalar_mul(whu[:], whsbs[g][:, j4, :],
                                                u2w2sb[:, jc, 0:1])
                    t1 = t1pool.tile([128, rows], BF16)
                    nc.scalar.activation(out=t1[:, 0:448], in_=v1b[:, 0:448],
                                         func=AF.Copy,
                                         scale=u2w2sb[:, jc, 1:2])
                    nc.vector.tensor_scalar_mul(t1[:, 448:rows],
                                                v1b[:, 448:rows],
                                                u2w2sb[:, jc, 1:2])
                    t2 = t2pool.tile([128, rows], BF16)
                    nc.vector.tensor_max(t2[:], t1[:], u1b[:])
                    t3 = t3pool.tile([128, rows], BF16)
                    nc.gpsimd.tensor_mul(t3[:, 0:272], t2[:, 0:272],
                                         mts[g][:, j4, 0:272])
                    nc.vector.tensor_mul(t3[:, 272:rows], t2[:, 272:rows],
                                         mts[g][:, j4, 272:rows])
                    for ic in range(rch):
                        nc.tensor.matmul(
                            out=po[ic][:],
                            lhsT=t3[:, ic * 128:(ic + 1) * 128],
                            rhs=whu[:],
                            start=(jc == 0), stop=(jc == nch - 1))

                # ---- epilogue: normalize, elu, log_softmax, all [i, f] ----
                # Ln is batched into one op at the end so the Act engine's
                # function table only swaps once.
                with tc.tile_pool(name="ep", bufs=2) as ep:
                    elus = [consts.tile([128, fout], F32, name=f"elu{_i}")
                            for _i in range(rch)]
                    sms = consts.tile([128, rch], F32)
                    lnts = consts.tile([128, rch], F32)
                    for ic in range(rch):
                        rc = ep.tile([128, 1], F32, name="rc", tag="rc")
                        nc.vector.reciprocal(out=rc[:],
                                             in_=po[ic][:, fout:fout + 1])
                        an = ep.tile([128, fout], F32, name="an", tag="an")
                        nc.scalar.activation(out=an[:], in_=po[ic][:, 0:fout],
                                             func=AF.Copy, scale=rc[:])
                        mg = ep.tile([128, fout], F32, name="mg", tag="mg")
                        nc.vector.tensor_scalar(out=mg[:],
                                                in0=po[ic][:, 0:fout],
                                                scalar1=rc[:], scalar2=0.0,
                                                op0=OP.mult, op1=OP.min)
                        em = ep.tile([128, fout], F32, name="em", tag="em")
                        nc.scalar.activation(out=em[:], in_=mg[:], func=AF.Exp)
                        nc.vector.scalar_tensor_tensor(
                            out=elus[ic][:], in0=em[:], scalar=-1.0, in1=an[:],
                            op0=OP.add, op1=OP.max)
                        # elu <= ~6 so exp cannot overflow f32; skip
                        # the max-shift entirely
                        ex = ep.tile([128, fout], F32, name="ex", tag="ex")
                        nc.scalar.activation(out=ex[:], in_=elus[ic][:],
                                             func=AF.Exp,
                                             accum_out=sms[:, ic:ic + 1])
                    nc.scalar.activation(out=lnts[:], in_=sms[:], func=AF.Ln)
                    fins = consts.tile([128, rch, fout], F32)
                    for ic in range(rch):
                        nc.vector.tensor_scalar_sub(fins[:, ic, :],
                                                    elus[ic][:],
                                                    lnts[:, ic:ic + 1])
                    nc.sync.dma_start(
                        out=bass.AP(tensor=out, offset=0,
                                    ap=[[fout, 128], [128 * fout, rch],
                                        [1, fout]]),
                        in_=fins[:])

    nc.compile()
    return nc


_CACHE = {}


def _get_programs():
    if "p1" not in _CACHE:
        _CACHE["p1"] = build_phase1()
        _CACHE["p2"] = build_phase2()
    return _CACHE["p1"], _CACHE["p2"]


def prep_phase1_inputs(x, adj, W_heads, a1_heads, a2_heads, W_out, a1_out,
                       a2_out):
    bf = ml_dtypes.bfloat16
    nch = N // 128
    xT = np.ascontiguousarray(x.T).astype(bf)
    maskT = np.ascontiguousarray((adj > 0).T.astype(np.float32)).astype(bf)
    in1 = []
    for h in range(NCORES):
        f1 = x @ (W_heads[h] @ a1_heads[h])
        f2 = x @ (W_heads[h] @ a2_heads[h])
        u1v1 = np.stack([np.exp(f1), np.exp(ALPHA * f1)], axis=0).astype(bf)
        u2w2 = np.stack(
            [np.exp(f2).reshape(nch, 128).T,
             np.exp(-0.8 * f2).reshape(nch, 128).T],
            axis=2).astype(np.float32)
        Wo = W_out[h * HID:(h + 1) * HID]
        wocat = np.concatenate(
            [Wo, (Wo @ a1_out)[:, None], (Wo @ a2_out)[:, None]],
            axis=1)
        wocat = np.concatenate([wocat, wocat.sum(0, keepdims=True)],
                               axis=0).astype(bf)
        in1.append({"xT": xT, "maskT": maskT,
                    "wcat": np.ascontiguousarray(W_heads[h].astype(bf)),
                    "wocat": np.ascontiguousarray(wocat),
                    "u1v1": np.ascontiguousarray(u1v1),
                    "u2w2": np.ascontiguousarray(u2w2)})
    return in1, maskT


def prep_phase2_inputs(r1_results, maskT):
    bf = ml_dtypes.bfloat16
    wh2full = np.zeros((N, FOUT + 2), np.float32)
    for h in range(NCORES):
        wh2full += r1_results[h]["wh2p"].astype(np.float32)
    f1o = wh2full[:, FOUT]
    f2o = wh2full[:, FOUT + 1]
    wh2in = np.concatenate(
        [wh2full[:, 0:FOUT], np.ones((N, 1), np.float32)],
        axis=1).astype(bf)
    nch = N // 128
    u2w2 = np.stack(
        [np.exp(f2o).reshape(nch, 128).T,
         np.exp(-0.8 * f2o).reshape(nch, 128).T],
        axis=2).astype(np.float32)
    rows = N // NCORES
    in2 = []
    for c in range(NCORES):
        rs = slice(c * rows, (c + 1) * rows)
        u1v2 = np.stack([np.exp(f1o[rs]), np.exp(ALPHA * f1o[rs])],
                        axis=0).astype(bf)
        in2.append({"wh2": wh2in,
                    "m2": np.ascontiguousarray(maskT[:, rs]),
                    "u1v2": u1v2,
                    "u2w2": np.ascontiguousarray(u2w2)})
    return in2


def kernel(x, adj, W_heads, a1_heads, a2_heads, W_out, a1_out, a2_out, **_):
    x = np.asarray(x, dtype=np.float32)
    adj = np.asarray(adj)
    W_heads = np.asarray(W_heads, dtype=np.float32)
    a1_heads = np.asarray(a1_heads, dtype=np.float32)
    a2_heads = np.asarray(a2_heads, dtype=np.float32)
    W_out = np.asarray(W_out, dtype=np.float32)
    a1_out = np.asarray(a1_out, dtype=np.float32)
    a2_out = np.asarray(a2_out, dtype=np.float32)

    p1, p2 = _get_programs()
    in1, maskT = prep_phase1_inputs(x, adj, W_heads, a1_heads, a2_heads,
                                    W_out, a1_out, a2_out)
    r1 = run_bass_kernel_spmd(p1, in1, core_ids=list(range(NCORES))).results
    in2 = prep_phase2_inputs(r1, maskT)
    r2 = run_bass_kernel_spmd(p2, in2, core_ids=list(range(NCORES))).results
    out = np.concatenate([r2[c]["out"] for c in range(NCORES)], axis=0)
    return out.astype(np.float32)



# revision 2
# speedup vs baseline: 1.7803x; 1.7803x over previous
"""GAT (2-layer, 8-head) Trainium2 kernel, 8-core SPMD — sort-based layer 1.

Layer 1 (head-parallel, one head per core) avoids materializing the [N,N]
score matrix entirely.  With g = f1_i + f2_j and p = exp(leakyrelu(g))*m:
    p = A[i]*q2'[j]*m          when g < 0   (A = e^{0.2 f1},  q2' = e^{0.2 f2})
    p = B[i]*u2'[j]*m          when g >= 0  (B = e^{f1},      u2' = e^{f2})
Sorting j by f2 and i by the cutoff c(i) = #{j : f2_j < -f1_i} makes the
branch a block predicate: for column-block k (128 sorted i's) and row-chunk
q (128 sorted j's), q < b_lo(k) is pure branch-1, q > b_hi(k) pure branch-2,
and the few boundary chunks are host-presplit into bd1/bd2 = m*1[branch].
The PE then consumes the raw permuted mask (fp8, exact for 0/1) as the
stationary operand and streams [Wh*q2'|q2'] / [Wh*u2'|u2'] (66 bf16 rows)
as the moving operand, accumulating S1/S2 per block in PSUM; the drain
combines num = A*S1 + B*S2 on Act/DVE.  No elementwise pass ever touches
an [N,N] tensor, so the phase is paced by the 16.7MB fp8 mask DMA.

Layer 2 (row-parallel, 512 rows per core) keeps the dense-score pipeline but
with scores p~ = max(q2o[j], r8[i]*u2o[j]) * m  (r8 = e^{0.8 f1o}; the
e^{0.2 f1_i} factor cancels in the softmax): one fused 2-scalar tensor_scalar
(DVE 4x mode) plus one mask multiply split DVE/Pool, then the attention
matmul with plain [Wh2|1] as the moving operand.

The host does the O(N*F) prep (projections, exp vectors, sorts, mask
permutations and boundary splits, inter-layer elu/concat) in numpy; both
bass programs are built at first kernel() call from the inputs' cutoff
structure (uniform across cores so the programs stay SPMD).
"""

import sys

for p in ("/opt/trn_rl_repo", "/opt/pypackages"):
    if p not in sys.path:
        sys.path.append(p)

import numpy as np
import ml_dtypes

import concourse.bass as bass
import concourse.bacc as bacc
import concourse.tile as tile
from concourse import mybir
from concourse.bass_utils import run_bass_kernel_spmd

BF16 = mybir.dt.bfloat16
FP8 = mybir.dt.float8e4
F32 = mybir.dt.float32
OP = mybir.AluOpType
AF = mybir.ActivationFunctionType

NPBF = ml_dtypes.bfloat16
NPF8 = ml_dtypes.float8_e4m3fn

N, FIN, HID, HEADS, FOUT = 4096, 512, 64, 8, 256
NCORES = 8
NCH = N // 128          # 32 row chunks / column blocks
WC1 = HID + 2           # 64 features | den | pad
WC2 = FOUT + 1          # 256 features | den
ROWS2 = N // NCORES     # 512 output rows per core in layer 2
ALPHA = 0.2


# --------------------------------------------------------------------------
# phase 1: sort-based head attention.  spans = ((b_lo, b_hi), ...) per block,
# uniform across heads; bdoff[k] = offset of block k's boundary tiles.
# --------------------------------------------------------------------------
def build_phase1(spans):
    nc = bacc.Bacc("TRN2", target_bir_lowering=False, debug=False,
                   enable_asserts=False)
    sb = sum(hi - lo + 1 for lo, hi in spans)
    bdoff = []
    off = 0
    for lo, hi in spans:
        bdoff.append(off)
        off += hi - lo + 1

    # slab r holds mask columns [256r, 256r+256) for all 32 row chunks,
    # host-packed contiguous per partition: [p][q][c] with c in-block col
    mps = nc.dram_tensor("mps", [16, 128, NCH * 256], FP8,
                         kind="ExternalInput")
    bd1 = nc.dram_tensor("bd1", [128, sb * 128], FP8, kind="ExternalInput")
    bd2 = nc.dram_tensor("bd2", [128, sb * 128], FP8, kind="ExternalInput")
    rqd = nc.dram_tensor("rqd", [128, NCH * WC1], BF16, kind="ExternalInput")
    rud = nc.dram_tensor("rud", [128, NCH * WC1], BF16, kind="ExternalInput")
    abd = nc.dram_tensor("abd", [128, NCH * 2], F32, kind="ExternalInput")
    numo = nc.dram_tensor("numo", [128, NCH * WC1], BF16,
                          kind="ExternalOutput")

    with tile.TileContext(nc) as tc:
        with tc.tile_pool(name="consts", bufs=1) as consts:
            rq = consts.tile([128, NCH * WC1], BF16)
            ru = consts.tile([128, NCH * WC1], BF16)
            ab = consts.tile([128, NCH * 2], F32)
            bd1s = consts.tile([128, sb * 128], FP8)
            bd2s = consts.tile([128, sb * 128], FP8)
            stage = consts.tile([128, NCH * WC1], BF16)
            nc.gpsimd.dma_start(out=rq[:], in_=rqd[:, :])
            nc.gpsimd.dma_start(out=ru[:], in_=rud[:, :])
            nc.gpsimd.dma_start(out=ab[:], in_=abd[:, :])
            nc.scalar.dma_start(out=bd1s[:], in_=bd1[:, :])
            nc.scalar.dma_start(out=bd2s[:], in_=bd2[:, :])
            with (
                tc.tile_pool(name="slabs", bufs=2) as slabs,
                tc.tile_pool(name="ps1", bufs=4, space="PSUM") as ps1p,
                tc.tile_pool(name="ps2", bufs=4, space="PSUM") as ps2p,
                tc.tile_pool(name="ep", bufs=4) as ep,
            ):
                for r in range(16):          # 2 column blocks per round
                    slab = slabs.tile([128, NCH * 256], FP8, name="slab",
                                      tag="slab")
                    nc.sync.dma_start(
                        out=slab[:],
                        in_=bass.AP(tensor=mps, offset=r * 128 * NCH * 256,
                                    ap=[[NCH * 256, 128], [1, NCH * 256]]))
                    ks = (2 * r, 2 * r + 1)
                    ps1 = {k: ps1p.tile([128, WC1], F32, name=f"ps1_{k}",
                                        tag="ps1") for k in ks}
                    ps2 = {k: ps2p.tile([128, WC1], F32, name=f"ps2_{k}",
                                        tag="ps2") for k in ks}
                    for q in range(NCH):
                        for k in ks:
                            lo, hi = spans[k]
                            col = q * 256 + (k % 2) * 128
                            rqs = rq[:, q * WC1:(q + 1) * WC1]
                            rus = ru[:, q * WC1:(q + 1) * WC1]
                            if q < lo:
                                nc.tensor.matmul(
                                    out=ps1[k][:],
                                    lhsT=slab[:, col:col + 128],
                                    rhs=rqs, start=(q == 0), stop=False)
                            elif q > hi:
                                nc.tensor.matmul(
                                    out=ps2[k][:],
                                    lhsT=slab[:, col:col + 128],
                                    rhs=rus, start=False, stop=(q == NCH - 1))
                            else:
                                i0 = (bdoff[k] + q - lo) * 128
                                nc.tensor.matmul(
                                    out=ps1[k][:],
                                    lhsT=bd1s[:, i0:i0 + 128],
                                    rhs=rqs, start=(q == 0), stop=(q == hi))
                                nc.tensor.matmul(
                                    out=ps2[k][:],
                                    lhsT=bd2s[:, i0:i0 + 128],
                                    rhs=rus, start=(q == lo),
                                    stop=(q == NCH - 1))
                    for k in ks:
                        t1 = ep.tile([128, WC1], BF16, name="t1", tag="t1")
                        nc.scalar.activation(out=t1[:], in_=ps1[k][:],
                                             func=AF.Copy,
                                             scale=ab[:, 2 * k:2 * k + 1])
                        nc.vector.scalar_tensor_tensor(
                            out=stage[:, k * WC1:(k + 1) * WC1],
                            in0=ps2[k][:], scalar=ab[:, 2 * k + 1:2 * k + 2],
                            in1=t1[:], op0=OP.mult, op1=OP.add)
            nc.sync.dma_start(out=numo[:, :], in_=stage[:])

    nc.compile()
    return nc


# --------------------------------------------------------------------------
# phase 2: dense-score layer-2 attention for 512 rows per core.
# --------------------------------------------------------------------------
def build_phase2():
    nc = bacc.Bacc("TRN2", target_bir_lowering=False, debug=False,
                   enable_asserts=False)
    rows = ROWS2
    rch = rows // 128
    DVE_SPLIT = 384      # t3 cols on DVE; rest on Pool

    wh2 = nc.dram_tensor("wh2", [128, NCH * WC2], BF16, kind="ExternalInput")
    m2 = nc.dram_tensor("m2", [128, NCH * rows], BF16, kind="ExternalInput")
    r8d = nc.dram_tensor("r8d", [1, rows], BF16, kind="ExternalInput")
    qud = nc.dram_tensor("qud", [128, NCH * 2], F32, kind="ExternalInput")
    out = nc.dram_tensor("out", [rows, FOUT], F32, kind="ExternalOutput")

    with tile.TileContext(nc) as tc:
        with tc.tile_pool(name="consts", bufs=1) as consts:
            qu = consts.tile([128, NCH * 2], F32)
            nc.sync.dma_start(out=qu[:], in_=qud[:, :])
            r8b = consts.tile([128, rows], BF16)
            nc.sync.dma_start(
                out=r8b[:],
                in_=bass.AP(tensor=r8d, offset=0, ap=[[0, 128], [1, rows]]))
            wh2sb = consts.tile([128, NCH * WC2], BF16)
            m2sb = consts.tile([128, NCH * rows], BF16)
            nc.gpsimd.dma_start(out=wh2sb[:], in_=wh2[:, :])
            nc.scalar.dma_start(out=m2sb[:, 0:16 * rows],
                                in_=m2[:, 0:16 * rows])
            nc.scalar.dma_start(out=m2sb[:, 16 * rows:NCH * rows],
                                in_=m2[:, 16 * rows:NCH * rows])

            with (
                tc.tile_pool(name="t2pool", bufs=3) as t2pool,
                tc.tile_pool(name="t3pool", bufs=3) as t3pool,
                tc.tile_pool(name="ps4", bufs=rch, space="PSUM") as ps4,
            ):
                po = [ps4.tile([128, WC2], F32, name=f"po{_i}", tag="po")
                      for _i in range(rch)]
                for jc in range(NCH):
                    t2 = t2pool.tile([128, rows], BF16)
                    nc.vector.tensor_scalar(
                        out=t2[:], in0=r8b[:],
                        scalar1=qu[:, 2 * jc:2 * jc + 1],
                        scalar2=qu[:, 2 * jc + 1:2 * jc + 2],
                        op0=OP.mult, op1=OP.max)
                    t3 = t3pool.tile([128, rows], BF16)
                    ms = m2sb[:, jc * rows:(jc + 1) * rows]
                    nc.vector.tensor_mul(t3[:, 0:DVE_SPLIT], t2[:, 0:DVE_SPLIT],
                                         ms[:, 0:DVE_SPLIT])
                    nc.gpsimd.tensor_mul(t3[:, DVE_SPLIT:rows],
                                         t2[:, DVE_SPLIT:rows],
                                         ms[:, DVE_SPLIT:rows])
                    for ic in range(rch):
                        nc.tensor.matmul(
                            out=po[ic][:],
                            lhsT=t3[:, ic * 128:(ic + 1) * 128],
                            rhs=wh2sb[:, jc * WC2:(jc + 1) * WC2],
                            start=(jc == 0), stop=(jc == NCH - 1))

                # epilogue: normalize, elu, log_softmax in [i, f] layout
                with tc.tile_pool(name="ep", bufs=2) as ep:
                    elus = [consts.tile([128, FOUT], F32, name=f"elu{_i}")
                            for _i in range(rch)]
                    sms = consts.tile([128, rch], F32)
                    lnts = consts.tile([128, rch], F32)
                    for ic in range(rch):
                        rc = ep.tile([128, 1], F32, name="rc", tag="rc")
                        nc.vector.reciprocal(out=rc[:],
                                             in_=po[ic][:, FOUT:FOUT + 1])
                        an = ep.tile([128, FOUT], F32, name="an", tag="an")
                        nc.scalar.activation(out=an[:], in_=po[ic][:, 0:FOUT],
                                             func=AF.Copy, scale=rc[:])
                        mg = ep.tile([128, FOUT], F32, name="mg", tag="mg")
                        nc.vector.tensor_scalar(out=mg[:],
                                                in0=po[ic][:, 0:FOUT],
                                                scalar1=rc[:], scalar2=0.0,
                                                op0=OP.mult, op1=OP.min)
                        em = ep.tile([128, FOUT], F32, name="em", tag="em")
                        nc.scalar.activation(out=em[:], in_=mg[:], func=AF.Exp)
                        nc.vector.scalar_tensor_tensor(
                            out=elus[ic][:], in0=em[:], scalar=-1.0,
                            in1=an[:], op0=OP.add, op1=OP.max)
                        # elu <= ~6 so the next exp cannot overflow fp32
                        ex = ep.tile([128, FOUT], F32, name="ex", tag="ex")
                        nc.scalar.activation(out=ex[:], in_=elus[ic][:],
                                             func=AF.Exp,
                                             accum_out=sms[:, ic:ic + 1])
                    nc.scalar.activation(out=lnts[:], in_=sms[:], func=AF.Ln)
                    fins = consts.tile([128, rch, FOUT], F32)
                    for ic in range(rch):
                        nc.vector.tensor_scalar_sub(fins[:, ic, :],
                                                    elus[ic][:],
                                                    lnts[:, ic:ic + 1])
                    nc.sync.dma_start(
                        out=bass.AP(tensor=out, offset=0,
                                    ap=[[FOUT, 128], [128 * FOUT, rch],
                                        [1, FOUT]]),
                        in_=fins[:])

    nc.compile()
    return nc


_CACHE = {}


def _get_programs():
    return _CACHE["p1"], _CACHE["p2"]


# --------------------------------------------------------------------------
# host-side prep
# --------------------------------------------------------------------------
def _sort_structure(f1, f2):
    """sigma (rows by f2), tau (cols by cutoff), cutoffs c, block bounds."""
    sigma = np.argsort(f2, kind="stable")
    f2s = f2[sigma]
    c = np.searchsorted(f2s, -f1, side="left")   # branch-1 count per col
    tau = np.argsort(c, kind="stable")
    cs = c[tau]
    b = cs // 128                                 # boundary chunk per col
    lo = np.minimum(b.reshape(NCH, 128).min(axis=1), NCH - 1)
    hi = np.minimum(b.reshape(NCH, 128).max(axis=1), NCH - 1)
    return sigma, tau, cs, lo, hi


def prep_phase1(x, adj, W_heads, a1_heads, a2_heads):
    maskT8 = (adj > 0).T.astype(NPF8)            # maskT[j, i] = adj[i, j]
    mu8 = maskT8.view(np.uint8)

    heads = []
    los = np.full(NCH, NCH - 1, np.int64)
    his = np.zeros(NCH, np.int64)
    for h in range(HEADS):
        Wh = (x @ W_heads[h]).astype(np.float32)          # [N, 64]
        f1 = Wh @ a1_heads[h]
        f2 = Wh @ a2_heads[h]
        sigma, tau, cs, lo, hi = _sort_structure(f1, f2)
        los = np.minimum(los, lo)
        his = np.maximum(his, hi)
        heads.append((Wh, f1, f2, sigma, tau, cs))
    spans = tuple((int(l), int(h)) for l, h in zip(los, his))
    sb = sum(h - l + 1 for l, h in spans)

    in1 = []
    for h in range(HEADS):
        Wh, f1, f2, sigma, tau, cs = heads[h]
        mp = mu8[np.ix_(sigma, tau)]                      # [N, N] permuted
        # slabs: [16][p][q][256] = mp[128q + p, 256r + c]
        mps = np.ascontiguousarray(
            mp.reshape(NCH, 128, 16, 256).transpose(2, 1, 0, 3)
            .reshape(16, 128, NCH * 256)).view(NPF8)
        # boundary splits: bd[r, sbidx*128 + t] for block k, chunk q
        bd1 = np.zeros((128, sb * 128), np.uint8)
        bd2 = np.zeros((128, sb * 128), np.uint8)
        one = np.float32(1.0).astype(NPF8).view(np.uint8)
        idx = 0
        for k, (lo, hi) in enumerate(spans):
            ck = cs[k * 128:(k + 1) * 128]                # cutoffs, this block
            for q in range(lo, hi + 1):
                mblk = mp[q * 128:(q + 1) * 128, k * 128:(k + 1) * 128]
                r = np.arange(q * 128, q * 128 + 128)[:, None]
                br1 = (r < ck[None, :])
                bd1[:, idx * 128:(idx + 1) * 128] = np.where(br1, mblk, 0)
                bd2[:, idx * 128:(idx + 1) * 128] = np.where(br1, 0, mblk)
                idx += 1
        f2s = f2[sigma]
        q2p = np.exp(ALPHA * f2s).astype(np.float32)      # e^{0.2 f2}
        u2p = np.exp(f2s).astype(np.float32)              # e^{f2}
        Whs = Wh[sigma]                                   # sorted rows
        rqf = np.concatenate([Whs * q2p[:, None], q2p[:, None],
                              np.zeros((N, 1), np.float32)], axis=1)
        ruf = np.concatenate([Whs * u2p[:, None], u2p[:, None],
                              np.zeros((N, 1), np.float32)], axis=1)
        # [p][q][f] layout
        rqd = np.ascontiguousarray(
            rqf.reshape(NCH, 128, WC1).transpose(1, 0, 2)
            .reshape(128, NCH * WC1)).astype(NPBF)
        rud = np.ascontiguousarray(
            ruf.reshape(NCH, 128, WC1).transpose(1, 0, 2)
            .reshape(128, NCH * WC1)).astype(NPBF)
        f1t = f1[tau]
        abf = np.stack([np.exp(ALPHA * f1t), np.exp(f1t)],
                       axis=1).astype(np.float32)         # [N, 2] A|B
        abd = np.ascontiguousarray(
            abf.reshape(NCH, 128, 2).transpose(1, 0, 2)
            .reshape(128, NCH * 2))
        in1.append({"mps": mps, "bd1": bd1.view(NPF8), "bd2": bd2.view(NPF8),
                    "rqd": rqd, "rud": rud, "abd": abd})
    return in1, heads, spans


def finish_phase1(r1, heads):
    """num/den -> h (elu'd, concatenated) in natural node order."""
    H = np.zeros((N, HEADS * HID), np.float32)
    for h in range(HEADS):
        tau = heads[h][4]
        numo = r1[h]["numo"].astype(np.float32)           # [128, NCH*WC1]
        ns = numo.reshape(128, NCH, WC1).transpose(1, 0, 2).reshape(N, WC1)
        hv = ns[:, 0:HID] / ns[:, HID:HID + 1]            # sorted cols
        hn = np.empty_like(hv)
        hn[tau] = hv                                      # un-permute
        H[:, h * HID:(h + 1) * HID] = np.where(hn > 0, hn, np.expm1(hn))
    return H


def prep_phase2(H, adj, W_out, a1_out, a2_out):
    maskT = (adj > 0).T.astype(NPBF)
    Wh2 = (H @ W_out).astype(np.float32)                  # [N, 256]
    f1o = Wh2 @ a1_out
    f2o = Wh2 @ a2_out
    wh2f = np.concatenate([Wh2, np.ones((N, 1), np.float32)], axis=1)
    wh2d = np.ascontiguousarray(
        wh2f.reshape(NCH, 128, WC2).transpose(1, 0, 2)
        .reshape(128, NCH * WC2)).astype(NPBF)
    quf = np.stack([np.exp(f2o), np.exp(ALPHA * f2o)],
                   axis=1).astype(np.float32)             # u2o | q2o
    qud = np.ascontiguousarray(
        quf.reshape(NCH, 128, 2).transpose(1, 0, 2).reshape(128, NCH * 2))
    in2 = []
    for c in range(NCORES):
        rs = slice(c * ROWS2, (c + 1) * ROWS2)
        m2 = np.ascontiguousarray(
            maskT[:, rs].astype(np.float32).reshape(NCH, 128, ROWS2)
            .transpose(1, 0, 2).reshape(128, NCH * ROWS2)).astype(NPBF)
        r8 = np.exp(0.8 * f1o[rs]).astype(NPBF)[None, :]
        in2.append({"wh2": wh2d, "m2": m2, "r8d": r8, "qud": qud})
    return in2


def kernel(x, adj, W_heads, a1_heads, a2_heads, W_out, a1_out, a2_out, **_):
    x = np.asarray(x, dtype=np.float32)
    adj = np.asarray(adj)
    W_heads = np.asarray(W_heads, dtype=np.float32)
    a1_heads = np.asarray(a1_heads, dtype=np.float32)
    a2_heads = np.asarray(a2_heads, dtype=np.float32)
    W_out = np.asarray(W_out, dtype=np.float32)
    a1_out = np.asarray(a1_out, dtype=np.float32)
    a2_out = np.asarray(a2_out, dtype=np.float32)

    in1, heads, spans = prep_phase1(x, adj, W_heads, a1_heads, a2_heads)
    if _CACHE.get("spans") != spans:
        _CACHE["p1"] = build_phase1(spans)
        _CACHE["spans"] = spans
    if "p2" not in _CACHE:
        _CACHE["p2"] = build_phase2()
    p1, p2 = _CACHE["p1"], _CACHE["p2"]

    r1 = run_bass_kernel_spmd(p1, in1, core_ids=list(range(NCORES))).results
    H = finish_phase1(r1, heads)
    in2 = prep_phase2(H, adj, W_out, a1_out, a2_out)
    r2 = run_bass_kernel_spmd(p2, in2, core_ids=list(range(NCORES))).results
    out = np.concatenate([r2[c]["out"] for c in range(NCORES)], axis=0)
    return out.astype(np.float32)


# revision 10
# speedup vs baseline: 1.9771x; 1.1105x over previous
"""GAT (2-layer, 8-head) Trainium2 kernel, 8-core SPMD — sort-based layer 1.

Layer 1 (head-parallel, one head per core) avoids materializing the [N,N]
score matrix entirely.  With g = f1_i + f2_j and p = exp(leakyrelu(g))*m:
    p = A[i]*q2'[j]*m          when g < 0   (A = e^{0.2 f1},  q2' = e^{0.2 f2})
    p = B[i]*u2'[j]*m          when g >= 0  (B = e^{f1},      u2' = e^{f2})
Sorting j by f2 and i by the cutoff c(i) = #{j : f2_j < -f1_i} makes the
branch a block predicate: for column-block k (128 sorted i's) and row-chunk
q (128 sorted j's), q < b_lo(k) is pure branch-1, q > b_hi(k) pure branch-2,
and the few boundary chunks are host-presplit into bd1/bd2 = m*1[branch].
The PE then consumes the raw permuted mask (fp8, exact for 0/1) as the
stationary operand and streams [Wh*q2'|q2'] / [Wh*u2'|u2'] (66 bf16 rows)
as the moving operand, accumulating S1/S2 per block in PSUM; the drain
combines num = A*S1 + B*S2 on Act/DVE.  No elementwise pass ever touches
an [N,N] tensor, so the phase is paced by the 16.7MB fp8 mask DMA.

Layer 2 (row-parallel, 512 rows per core) keeps the dense-score pipeline but
with scores p~ = max(q2o[j], r8[i]*u2o[j]) * m  (r8 = e^{0.8 f1o}; the
e^{0.2 f1_i} factor cancels in the softmax): one fused 2-scalar tensor_scalar
(DVE 4x mode) plus one mask multiply split DVE/Pool, then the attention
matmul with plain [Wh2|1] as the moving operand.

The host does the O(N*F) prep (projections, exp vectors, sorts, mask
permutations and boundary splits, inter-layer elu/concat) in numpy; both
bass programs are built at first kernel() call from the inputs' cutoff
structure (uniform across cores so the programs stay SPMD).
"""

import sys

for p in ("/opt/trn_rl_repo", "/opt/pypackages"):
    if p not in sys.path:
        sys.path.append(p)

import numpy as np
import ml_dtypes

import concourse.bass as bass
import concourse.bacc as bacc
import concourse.tile as tile
from concourse import mybir
from concourse.bass_utils import run_bass_kernel_spmd

BF16 = mybir.dt.bfloat16
FP8 = mybir.dt.float8e4
F32 = mybir.dt.float32
OP = mybir.AluOpType
AF = mybir.ActivationFunctionType

NPBF = ml_dtypes.bfloat16
NPF8 = ml_dtypes.float8_e4m3fn

N, FIN, HID, HEADS, FOUT = 4096, 512, 64, 8, 256
NCORES = 8
NCH = N // 128          # 32 row chunks / column blocks
WC1 = HID + 2           # 64 features | den | pad
WC2 = FOUT + 1          # 256 features | den
ROWS2 = N // NCORES     # 512 output rows per core in layer 2
ALPHA = 0.2


# --------------------------------------------------------------------------
# phase 1: sort-based head attention.  spans = ((b_lo, b_hi), ...) per block,
# uniform across heads; bdoff[k] = offset of block k's boundary tiles.
# --------------------------------------------------------------------------
def build_phase1(spans):
    nc = bacc.Bacc("TRN2", target_bir_lowering=False, debug=False,
                   enable_asserts=False)
    sb = sum(hi - lo + 1 for lo, hi in spans)
    bdoff = []
    off = 0
    for lo, hi in spans:
        bdoff.append(off)
        off += hi - lo + 1

    # slab r holds mask columns [256r, 256r+256) for all 32 row chunks,
    # host-packed contiguous per partition: [p][q][c] with c in-block col.
    # Boundary-chunk blocks of the slab are pre-split by the host to the
    # branch-2 part (bd2); the branch-1 part ships separately as bd1.
    mps = nc.dram_tensor("mps", [16, 128, NCH * 256], FP8,
                         kind="ExternalInput")
    bd1 = nc.dram_tensor("bd1", [128, sb * 128], FP8, kind="ExternalInput")
    rqd = nc.dram_tensor("rqd", [128, NCH * WC1], BF16, kind="ExternalInput")
    rud = nc.dram_tensor("rud", [128, NCH * WC1], BF16, kind="ExternalInput")
    abd = nc.dram_tensor("abd", [128, NCH * 2], F32, kind="ExternalInput")
    numo = nc.dram_tensor("numo", [128, NCH * WC1], BF16,
                          kind="ExternalOutput")

    with tile.TileContext(nc) as tc:
        with tc.tile_pool(name="consts", bufs=1) as consts:
            rq = consts.tile([128, NCH * WC1], BF16)
            ru = consts.tile([128, NCH * WC1], BF16)
            ab = consts.tile([128, NCH * 2], F32)
            bd1s = consts.tile([128, sb * 128], FP8)
            stage = consts.tile([128, NCH * WC1], BF16)
            nc.gpsimd.dma_start(out=rq[:], in_=rqd[:, :])
            nc.gpsimd.dma_start(out=ru[:], in_=rud[:, :])
            nc.gpsimd.dma_start(out=ab[:], in_=abd[:, :])
            bq = (sb + 3) // 4 * 128
            for i in range(4):
                c0, c1 = i * bq, min((i + 1) * bq, sb * 128)
                if c0 < c1:
                    nc.scalar.dma_start(out=bd1s[:, c0:c1],
                                        in_=bd1[:, c0:c1])
            with (
                tc.tile_pool(name="slabs", bufs=2) as slabs,
                tc.tile_pool(name="ps1", bufs=4, space="PSUM") as ps1p,
                tc.tile_pool(name="ps2", bufs=4, space="PSUM") as ps2p,
                tc.tile_pool(name="ep", bufs=4) as ep,
            ):
                for r in range(16):          # 2 column blocks per round
                    slab = slabs.tile([128, NCH * 256], FP8, name="slab",
                                      tag="slab")
                    eng = nc.sync if r % 2 == 0 else nc.gpsimd
                    eng.dma_start(
                        out=slab[:],
                        in_=bass.AP(tensor=mps, offset=r * 128 * NCH * 256,
                                    ap=[[NCH * 256, 128], [1, NCH * 256]]))
                    ks = (2 * r, 2 * r + 1)
                    ps1 = {k: ps1p.tile([128, WC1], F32, name=f"ps1_{k}",
                                        tag="ps1") for k in ks}
                    ps2 = {k: ps2p.tile([128, WC1], F32, name=f"ps2_{k}",
                                        tag="ps2") for k in ks}
                    for q in range(NCH):
                        for k in ks:
                            lo, hi = spans[k]
                            col = q * 256 + (k % 2) * 128
                            rqs = rq[:, q * WC1:(q + 1) * WC1]
                            rus = ru[:, q * WC1:(q + 1) * WC1]
                            if q < lo:
                                nc.tensor.matmul(
                                    out=ps1[k][:],
                                    lhsT=slab[:, col:col + 128],
                                    rhs=rqs, start=(q == 0), stop=False)
                            elif q > hi:
                                nc.tensor.matmul(
                                    out=ps2[k][:],
                                    lhsT=slab[:, col:col + 128],
                                    rhs=rus, start=False, stop=(q == NCH - 1))
                            else:
                                i0 = (bdoff[k] + q - lo) * 128
                                nc.tensor.matmul(
                                    out=ps1[k][:],
                                    lhsT=bd1s[:, i0:i0 + 128],
                                    rhs=rqs, start=(q == 0), stop=(q == hi))
                                nc.tensor.matmul(
                                    out=ps2[k][:],
                                    lhsT=slab[:, col:col + 128],
                                    rhs=rus, start=(q == lo),
                                    stop=(q == NCH - 1))
                    for k in ks:
                        t1 = ep.tile([128, WC1], BF16, name="t1", tag="t1")
                        nc.scalar.activation(out=t1[:], in_=ps1[k][:],
                                             func=AF.Copy,
                                             scale=ab[:, 2 * k:2 * k + 1])
                        nc.vector.scalar_tensor_tensor(
                            out=stage[:, k * WC1:(k + 1) * WC1],
                            in0=ps2[k][:], scalar=ab[:, 2 * k + 1:2 * k + 2],
                            in1=t1[:], op0=OP.mult, op1=OP.add)
            nc.sync.dma_start(out=numo[:, :], in_=stage[:])

    nc.compile()
    return nc


# --------------------------------------------------------------------------
# phase 2: dense-score layer-2 attention for 512 rows per core.
# --------------------------------------------------------------------------
def build_phase2():
    nc = bacc.Bacc("TRN2", target_bir_lowering=False, debug=False,
                   enable_asserts=False)
    rows = ROWS2
    rch = rows // 128
    DVE_SPLIT = 320      # t3 cols on DVE (1x, fp8 mask); rest on Pool

    wh2 = nc.dram_tensor("wh2", [128, NCH * WC2], BF16, kind="ExternalInput")
    m2 = nc.dram_tensor("m2", [128, NCH * rows], FP8, kind="ExternalInput")
    r8d = nc.dram_tensor("r8d", [1, rows], BF16, kind="ExternalInput")
    qud = nc.dram_tensor("qud", [128, NCH * 2], F32, kind="ExternalInput")
    out = nc.dram_tensor("out", [rows, FOUT], F32, kind="ExternalOutput")

    with tile.TileContext(nc) as tc:
        with tc.tile_pool(name="consts", bufs=1) as consts:
            qu = consts.tile([128, NCH * 2], F32)
            nc.sync.dma_start(out=qu[:], in_=qud[:, :])
            r8b = consts.tile([128, rows], BF16)
            nc.sync.dma_start(
                out=r8b[:],
                in_=bass.AP(tensor=r8d, offset=0, ap=[[0, 128], [1, rows]]))
            wh2sb = consts.tile([128, NCH * WC2], BF16)
            m2sb = consts.tile([128, NCH * rows], FP8)
            for i in range(4):
                nc.gpsimd.dma_start(
                    out=wh2sb[:, i * 8 * WC2:(i + 1) * 8 * WC2],
                    in_=wh2[:, i * 8 * WC2:(i + 1) * 8 * WC2])
                nc.scalar.dma_start(
                    out=m2sb[:, i * 8 * rows:(i + 1) * 8 * rows],
                    in_=m2[:, i * 8 * rows:(i + 1) * 8 * rows])

            with (
                tc.tile_pool(name="t2pool", bufs=3) as t2pool,
                tc.tile_pool(name="t3pool", bufs=3) as t3pool,
                tc.tile_pool(name="ps4", bufs=rch, space="PSUM") as ps4,
            ):
                po = [ps4.tile([128, WC2], F32, name=f"po{_i}", tag="po")
                      for _i in range(rch)]
                for jc in range(NCH):
                    t2 = t2pool.tile([128, rows], BF16)
                    nc.vector.tensor_scalar(
                        out=t2[:], in0=r8b[:],
                        scalar1=qu[:, 2 * jc:2 * jc + 1],
                        scalar2=qu[:, 2 * jc + 1:2 * jc + 2],
                        op0=OP.mult, op1=OP.max)
                    t3 = t3pool.tile([128, rows], BF16)
                    ms = m2sb[:, jc * rows:(jc + 1) * rows]
                    nc.vector.tensor_mul(t3[:, 0:DVE_SPLIT], t2[:, 0:DVE_SPLIT],
                                         ms[:, 0:DVE_SPLIT])
                    nc.gpsimd.tensor_mul(t3[:, DVE_SPLIT:rows],
                                         t2[:, DVE_SPLIT:rows],
                                         ms[:, DVE_SPLIT:rows])
                    for ic in range(rch):
                        nc.tensor.matmul(
                            out=po[ic][:],
                            lhsT=t3[:, ic * 128:(ic + 1) * 128],
                            rhs=wh2sb[:, jc * WC2:(jc + 1) * WC2],
                            start=(jc == 0), stop=(jc == NCH - 1))

                # epilogue: normalize, elu, log_softmax in [i, f] layout
                with tc.tile_pool(name="ep", bufs=2) as ep:
                    elus = [consts.tile([128, FOUT], F32, name=f"elu{_i}")
                            for _i in range(rch)]
                    sms = consts.tile([128, rch], F32)
                    lnts = consts.tile([128, rch], F32)
                    for ic in range(rch):
                        rc = ep.tile([128, 1], F32, name="rc", tag="rc")
                        nc.vector.reciprocal(out=rc[:],
                                             in_=po[ic][:, FOUT:FOUT + 1])
                        an = ep.tile([128, FOUT], F32, name="an", tag="an")
                        nc.scalar.activation(out=an[:], in_=po[ic][:, 0:FOUT],
                                             func=AF.Copy, scale=rc[:])
                        mg = ep.tile([128, FOUT], F32, name="mg", tag="mg")
                        nc.vector.tensor_scalar(out=mg[:],
                                                in0=po[ic][:, 0:FOUT],
                                                scalar1=rc[:], scalar2=0.0,
                                                op0=OP.mult, op1=OP.min)
                        em = ep.tile([128, FOUT], F32, name="em", tag="em")
                        nc.scalar.activation(out=em[:], in_=mg[:], func=AF.Exp)
                        nc.vector.scalar_tensor_tensor(
                            out=elus[ic][:], in0=em[:], scalar=-1.0,
                            in1=an[:], op0=OP.add, op1=OP.max)
                        # elu <= ~6 so the next exp cannot overflow fp32
                        ex = ep.tile([128, FOUT], F32, name="ex", tag="ex")
                        nc.scalar.activation(out=ex[:], in_=elus[ic][:],
                                             func=AF.Exp,
                                             accum_out=sms[:, ic:ic + 1])
                    nc.scalar.activation(out=lnts[:], in_=sms[:], func=AF.Ln)
                    fins = consts.tile([128, rch, FOUT], F32)
                    for ic in range(rch):
                        nc.vector.tensor_scalar_sub(fins[:, ic, :],
                                                    elus[ic][:],
                                                    lnts[:, ic:ic + 1])
                    nc.sync.dma_start(
                        out=bass.AP(tensor=out, offset=0,
                                    ap=[[FOUT, 128], [128 * FOUT, rch],
                                        [1, FOUT]]),
                        in_=fins[:])

    nc.compile()
    return nc


_CACHE = {}


def _get_programs():
    return _CACHE["p1"], _CACHE["p2"]


# --------------------------------------------------------------------------
# host-side prep
# --------------------------------------------------------------------------
def _sort_structure(f1, f2):
    """sigma (rows by f2), tau (cols by cutoff), cutoffs c, block bounds."""
    sigma = np.argsort(f2, kind="stable")
    f2s = f2[sigma]
    c = np.searchsorted(f2s, -f1, side="left")   # branch-1 count per col
    tau = np.argsort(c, kind="stable")
    cs = c[tau]
    b = cs // 128                                 # boundary chunk per col
    lo = np.minimum(b.reshape(NCH, 128).min(axis=1), NCH - 1)
    hi = np.minimum(b.reshape(NCH, 128).max(axis=1), NCH - 1)
    return sigma, tau, cs, lo, hi


def prep_phase1(x, adj, W_heads, a1_heads, a2_heads):
    maskT8 = (adj > 0).T.astype(NPF8)            # maskT[j, i] = adj[i, j]
    mu8 = maskT8.view(np.uint8)

    heads = []
    los = np.full(NCH, NCH - 1, np.int64)
    his = np.zeros(NCH, np.int64)
    for h in range(HEADS):
        Wh = (x @ W_heads[h]).astype(np.float32)          # [N, 64]
        f1 = Wh @ a1_heads[h]
        f2 = Wh @ a2_heads[h]
        sigma, tau, cs, lo, hi = _sort_structure(f1, f2)
        los = np.minimum(los, lo)
        his = np.maximum(his, hi)
        heads.append((Wh, f1, f2, sigma, tau, cs))
    spans = tuple((int(l), int(h)) for l, h in zip(los, his))
    sb = sum(h - l + 1 for l, h in spans)

    in1 = []
    for h in range(HEADS):
        Wh, f1, f2, sigma, tau, cs = heads[h]
        mp = mu8[np.ix_(sigma, tau)].copy()               # [N, N] permuted
        # boundary splits: bd1 ships separately; the branch-2 half
        # overwrites the boundary blocks of mp (consumed via the slab)
        bd1 = np.zeros((128, sb * 128), np.uint8)
        idx = 0
        for k, (lo, hi) in enumerate(spans):
            ck = cs[k * 128:(k + 1) * 128]                # cutoffs, this block
            for q in range(lo, hi + 1):
                mblk = mp[q * 128:(q + 1) * 128, k * 128:(k + 1) * 128]
                r = np.arange(q * 128, q * 128 + 128)[:, None]
                br1 = (r < ck[None, :])
                bd1[:, idx * 128:(idx + 1) * 128] = np.where(br1, mblk, 0)
                mp[q * 128:(q + 1) * 128,
                   k * 128:(k + 1) * 128] = np.where(br1, 0, mblk)
                idx += 1
        # slabs: [16][p][q][256] = mp[128q + p, 256r + c]
        mps = np.ascontiguousarray(
            mp.reshape(NCH, 128, 16, 256).transpose(2, 1, 0, 3)
            .reshape(16, 128, NCH * 256)).view(NPF8)
        f2s = f2[sigma]
        q2p = np.exp(ALPHA * f2s).astype(np.float32)      # e^{0.2 f2}
        u2p = np.exp(f2s).astype(np.float32)              # e^{f2}
        Whs = Wh[sigma]                                   # sorted rows
        rqf = np.concatenate([Whs * q2p[:, None], q2p[:, None],
                              np.zeros((N, 1), np.float32)], axis=1)
        ruf = np.concatenate([Whs * u2p[:, None], u2p[:, None],
                              np.zeros((N, 1), np.float32)], axis=1)
        # [p][q][f] layout
        rqd = np.ascontiguousarray(
            rqf.reshape(NCH, 128, WC1).transpose(1, 0, 2)
            .reshape(128, NCH * WC1)).astype(NPBF)
        rud = np.ascontiguousarray(
            ruf.reshape(NCH, 128, WC1).transpose(1, 0, 2)
            .reshape(128, NCH * WC1)).astype(NPBF)
        f1t = f1[tau]
        abf = np.stack([np.exp(ALPHA * f1t), np.exp(f1t)],
                       axis=1).astype(np.float32)         # [N, 2] A|B
        abd = np.ascontiguousarray(
            abf.reshape(NCH, 128, 2).transpose(1, 0, 2)
            .reshape(128, NCH * 2))
        in1.append({"mps": mps, "bd1": bd1.view(NPF8),
                    "rqd": rqd, "rud": rud, "abd": abd})
    return in1, heads, spans


def finish_phase1(r1, heads):
    """num/den -> h (elu'd, concatenated) in natural node order."""
    H = np.zeros((N, HEADS * HID), np.float32)
    for h in range(HEADS):
        tau = heads[h][4]
        numo = r1[h]["numo"].astype(np.float32)           # [128, NCH*WC1]
        ns = numo.reshape(128, NCH, WC1).transpose(1, 0, 2).reshape(N, WC1)
        hv = ns[:, 0:HID] / ns[:, HID:HID + 1]            # sorted cols
        hn = np.empty_like(hv)
        hn[tau] = hv                                      # un-permute
        H[:, h * HID:(h + 1) * HID] = np.where(hn > 0, hn, np.expm1(hn))
    return H


def prep_phase2(H, adj, W_out, a1_out, a2_out):
    maskT8 = (adj > 0).T.astype(NPF8)
    Wh2 = (H @ W_out).astype(np.float32)                  # [N, 256]
    f1o = Wh2 @ a1_out
    f2o = Wh2 @ a2_out
    wh2f = np.concatenate([Wh2, np.ones((N, 1), np.float32)], axis=1)
    wh2d = np.ascontiguousarray(
        wh2f.reshape(NCH, 128, WC2).transpose(1, 0, 2)
        .reshape(128, NCH * WC2)).astype(NPBF)
    quf = np.stack([np.exp(f2o), np.exp(ALPHA * f2o)],
                   axis=1).astype(np.float32)             # u2o | q2o
    qud = np.ascontiguousarray(
        quf.reshape(NCH, 128, 2).transpose(1, 0, 2).reshape(128, NCH * 2))
    in2 = []
    for c in range(NCORES):
        rs = slice(c * ROWS2, (c + 1) * ROWS2)
        m2 = np.ascontiguousarray(
            maskT8.reshape(NCH, 128, N)[:, :, rs]
            .transpose(1, 0, 2).reshape(128, NCH * ROWS2))
        r8 = np.exp(0.8 * f1o[rs]).astype(NPBF)[None, :]
        in2.append({"wh2": wh2d, "m2": m2, "r8d": r8, "qud": qud})
    return in2


def kernel(x, adj, W_heads, a1_heads, a2_heads, W_out, a1_out, a2_out, **_):
    x = np.asarray(x, dtype=np.float32)
    adj = np.asarray(adj)
    W_heads = np.asarray(W_heads, dtype=np.float32)
    a1_heads = np.asarray(a1_heads, dtype=np.float32)
    a2_heads = np.asarray(a2_heads, dtype=np.float32)
    W_out = np.asarray(W_out, dtype=np.float32)
    a1_out = np.asarray(a1_out, dtype=np.float32)
    a2_out = np.asarray(a2_out, dtype=np.float32)

    in1, heads, spans = prep_phase1(x, adj, W_heads, a1_heads, a2_heads)
    if _CACHE.get("spans") != spans:
        _CACHE["p1"] = build_phase1(spans)
        _CACHE["spans"] = spans
    if "p2" not in _CACHE:
        _CACHE["p2"] = build_phase2()
    p1, p2 = _CACHE["p1"], _CACHE["p2"]

    r1 = run_bass_kernel_spmd(p1, in1, core_ids=list(range(NCORES))).results
    H = finish_phase1(r1, heads)
    in2 = prep_phase2(H, adj, W_out, a1_out, a2_out)
    r2 = run_bass_kernel_spmd(p2, in2, core_ids=list(range(NCORES))).results
    out = np.concatenate([r2[c]["out"] for c in range(NCORES)], axis=0)
    return out.astype(np.float32)


# revision 14
# speedup vs baseline: 2.2047x; 1.1151x over previous
"""GAT (2-layer, 8-head) Trainium2 kernel, 8-core SPMD — sort-based layer 1.

Layer 1 (head-parallel, one head per core) avoids materializing the [N,N]
score matrix entirely.  With g = f1_i + f2_j and p = exp(leakyrelu(g))*m:
    p = A[i]*q2'[j]*m          when g < 0   (A = e^{0.2 f1},  q2' = e^{0.2 f2})
    p = B[i]*u2'[j]*m          when g >= 0  (B = e^{f1},      u2' = e^{f2})
Sorting j by f2 and i by the cutoff c(i) = #{j : f2_j < -f1_i} makes the
branch a block predicate: for column-block k (128 sorted i's) and row-chunk
q (128 sorted j's), q < b_lo(k) is pure branch-1, q > b_hi(k) pure branch-2,
and the few boundary chunks are host-presplit into bd1/bd2 = m*1[branch].
The PE then consumes the raw permuted mask (fp8, exact for 0/1) as the
stationary operand and streams [Wh*q2'|q2'] / [Wh*u2'|u2'] (66 bf16 rows)
as the moving operand, accumulating S1/S2 per block in PSUM; the drain
combines num = A*S1 + B*S2 on Act/DVE.  No elementwise pass ever touches
an [N,N] tensor, so the phase is paced by the 16.7MB fp8 mask DMA.

Layer 2 (row-parallel, 512 rows per core) keeps the dense-score pipeline but
with scores p~ = max(q2o[j], r8[i]*u2o[j]) * m  (r8 = e^{0.8 f1o}; the
e^{0.2 f1_i} factor cancels in the softmax): one fused 2-scalar tensor_scalar
(DVE 4x mode) plus one mask multiply split DVE/Pool, then the attention
matmul with plain [Wh2|1] as the moving operand.

The host does the O(N*F) prep (projections, exp vectors, sorts, mask
permutations and boundary splits, inter-layer elu/concat) in numpy; both
bass programs are built at first kernel() call from the inputs' cutoff
structure (uniform across cores so the programs stay SPMD).
"""

import sys

for p in ("/opt/trn_rl_repo", "/opt/pypackages"):
    if p not in sys.path:
        sys.path.append(p)

import numpy as np
import ml_dtypes

import concourse.bass as bass
import concourse.bacc as bacc
import concourse.tile as tile
from concourse import mybir
from concourse.bass_utils import run_bass_kernel_spmd

BF16 = mybir.dt.bfloat16
FP8 = mybir.dt.float8e4
F32 = mybir.dt.float32
OP = mybir.AluOpType
AF = mybir.ActivationFunctionType

NPBF = ml_dtypes.bfloat16
NPF8 = ml_dtypes.float8_e4m3fn

N, FIN, HID, HEADS, FOUT = 4096, 512, 64, 8, 256
NCORES = 8
NCH = N // 128          # 32 row chunks / column blocks
WC1 = HID + 2           # 64 features | den | pad
WC2 = FOUT + 1          # 256 features | den
ROWS2 = N // NCORES     # 512 output rows per core in layer 2
ALPHA = 0.2


# --------------------------------------------------------------------------
# phase 1: sort-based head attention.  spans = ((b_lo, b_hi), ...) per block,
# uniform across heads; bdoff[k] = offset of block k's boundary tiles.
# --------------------------------------------------------------------------
def build_phase1(spans):
    nc = bacc.Bacc("TRN2", target_bir_lowering=False, debug=False,
                   enable_asserts=False)
    sb = sum(hi - lo + 1 for lo, hi in spans)
    bdoff = []
    off = 0
    for lo, hi in spans:
        bdoff.append(off)
        off += hi - lo + 1

    # slab r holds mask columns [256r, 256r+256) for all 32 row chunks,
    # host-packed contiguous per partition: [p][q][c] with c in-block col.
    # Boundary-chunk blocks of the slab are pre-split by the host to the
    # branch-2 part (bd2); the branch-1 part ships separately as bd1.
    mps = nc.dram_tensor("mps", [16, 128, NCH * 256], FP8,
                         kind="ExternalInput")
    bd1 = nc.dram_tensor("bd1", [128, sb * 128], FP8, kind="ExternalInput")
    rqd = nc.dram_tensor("rqd", [128, NCH * WC1], BF16, kind="ExternalInput")
    rud = nc.dram_tensor("rud", [128, NCH * WC1], BF16, kind="ExternalInput")
    abd = nc.dram_tensor("abd", [128, NCH * 2], F32, kind="ExternalInput")
    numo = nc.dram_tensor("numo", [128, NCH * WC1], BF16,
                          kind="ExternalOutput")

    with tile.TileContext(nc) as tc:
        with tc.tile_pool(name="consts", bufs=1) as consts:
            rq = consts.tile([128, NCH * WC1], BF16)
            ru = consts.tile([128, NCH * WC1], BF16)
            ab = consts.tile([128, NCH * 2], F32)
            bd1s = consts.tile([128, sb * 128], FP8)
            stage = consts.tile([128, NCH * WC1], BF16)
            h1 = 8 * WC1
            nc.gpsimd.dma_start(out=rq[:, 0:h1], in_=rqd[:, 0:h1])
            nc.gpsimd.dma_start(out=ru[:, 0:h1], in_=rud[:, 0:h1])
            nc.gpsimd.dma_start(out=rq[:, h1:], in_=rqd[:, h1:])
            nc.gpsimd.dma_start(out=ru[:, h1:], in_=rud[:, h1:])
            nc.gpsimd.dma_start(out=ab[:], in_=abd[:, :])
            bq = (sb + 3) // 4 * 128
            for i in range(4):
                c0, c1 = i * bq, min((i + 1) * bq, sb * 128)
                if c0 < c1:
                    nc.scalar.dma_start(out=bd1s[:, c0:c1],
                                        in_=bd1[:, c0:c1])
            with (
                tc.tile_pool(name="slabs", bufs=2) as slabs,
                tc.tile_pool(name="ps1", bufs=4, space="PSUM") as ps1p,
                tc.tile_pool(name="ps2", bufs=4, space="PSUM") as ps2p,
                tc.tile_pool(name="ep", bufs=4) as ep,
            ):
                for r in range(16):          # 2 column blocks per round
                    slab = slabs.tile([128, NCH * 256], FP8, name="slab",
                                      tag="slab")
                    eng = nc.sync if r % 2 == 0 else nc.gpsimd
                    eng.dma_start(
                        out=slab[:],
                        in_=bass.AP(tensor=mps, offset=r * 128 * NCH * 256,
                                    ap=[[NCH * 256, 128], [1, NCH * 256]]))
                    ks = (2 * r, 2 * r + 1)
                    ps1 = {k: ps1p.tile([128, WC1], F32, name=f"ps1_{k}",
                                        tag="ps1") for k in ks}
                    ps2 = {k: ps2p.tile([128, WC1], F32, name=f"ps2_{k}",
                                        tag="ps2") for k in ks}
                    for q in range(NCH):
                        for k in ks:
                            lo, hi = spans[k]
                            col = q * 256 + (k % 2) * 128
                            rqs = rq[:, q * WC1:(q + 1) * WC1]
                            rus = ru[:, q * WC1:(q + 1) * WC1]
                            if q < lo:
                                nc.tensor.matmul(
                                    out=ps1[k][:],
                                    lhsT=slab[:, col:col + 128],
                                    rhs=rqs, start=(q == 0), stop=False)
                            elif q > hi:
                                nc.tensor.matmul(
                                    out=ps2[k][:],
                                    lhsT=slab[:, col:col + 128],
                                    rhs=rus, start=False, stop=(q == NCH - 1))
                            else:
                                i0 = (bdoff[k] + q - lo) * 128
                                nc.tensor.matmul(
                                    out=ps1[k][:],
                                    lhsT=bd1s[:, i0:i0 + 128],
                                    rhs=rqs, start=(q == 0), stop=(q == hi))
                                nc.tensor.matmul(
                                    out=ps2[k][:],
                                    lhsT=slab[:, col:col + 128],
                                    rhs=rus, start=(q == lo),
                                    stop=(q == NCH - 1))
                    for k in ks:
                        t1 = ep.tile([128, WC1], BF16, name="t1", tag="t1")
                        nc.scalar.activation(out=t1[:], in_=ps1[k][:],
                                             func=AF.Copy,
                                             scale=ab[:, 2 * k:2 * k + 1])
                        nc.vector.scalar_tensor_tensor(
                            out=stage[:, k * WC1:(k + 1) * WC1],
                            in0=ps2[k][:], scalar=ab[:, 2 * k + 1:2 * k + 2],
                            in1=t1[:], op0=OP.mult, op1=OP.add)
                    if r == 7:
                        nc.sync.dma_start(out=numo[:, 0:16 * WC1],
                                          in_=stage[:, 0:16 * WC1])
            nc.sync.dma_start(out=numo[:, 16 * WC1:], in_=stage[:, 16 * WC1:])

    nc.compile()
    return nc


# --------------------------------------------------------------------------
# phase 2: dense-score layer-2 attention for 512 rows per core.
# --------------------------------------------------------------------------
def build_phase2():
    nc = bacc.Bacc("TRN2", target_bir_lowering=False, debug=False,
                   enable_asserts=False)
    rows = ROWS2
    rch = rows // 128
    DVE_SPLIT = 320      # t3 cols on DVE (1x, fp8 mask); rest on Pool

    wh2 = nc.dram_tensor("wh2", [128, NCH * WC2], BF16, kind="ExternalInput")
    m2 = nc.dram_tensor("m2", [128, NCH * rows], FP8, kind="ExternalInput")
    r8d = nc.dram_tensor("r8d", [1, rows], BF16, kind="ExternalInput")
    qud = nc.dram_tensor("qud", [128, NCH * 2], F32, kind="ExternalInput")
    out = nc.dram_tensor("out", [128, rch * WC2], F32, kind="ExternalOutput")

    with tile.TileContext(nc) as tc:
        with tc.tile_pool(name="consts", bufs=1) as consts:
            qu = consts.tile([128, NCH * 2], F32)
            nc.sync.dma_start(out=qu[:], in_=qud[:, :])
            r8b = consts.tile([128, rows], BF16)
            nc.sync.dma_start(
                out=r8b[:],
                in_=bass.AP(tensor=r8d, offset=0, ap=[[0, 128], [1, rows]]))
            wh2sb = consts.tile([128, NCH * WC2], BF16)
            m2sb = consts.tile([128, NCH * rows], FP8)
            for i in range(8):
                nc.scalar.dma_start(
                    out=m2sb[:, i * 4 * rows:(i + 1) * 4 * rows],
                    in_=m2[:, i * 4 * rows:(i + 1) * 4 * rows])
                nc.gpsimd.dma_start(
                    out=wh2sb[:, i * 4 * WC2:(i + 1) * 4 * WC2],
                    in_=wh2[:, i * 4 * WC2:(i + 1) * 4 * WC2])

            with (
                tc.tile_pool(name="t2pool", bufs=6) as t2pool,
                tc.tile_pool(name="t3pool", bufs=6) as t3pool,
                tc.tile_pool(name="ps4", bufs=rch, space="PSUM") as ps4,
            ):
                po = [ps4.tile([128, WC2], F32, name=f"po{_i}", tag="po")
                      for _i in range(rch)]
                for jc in range(NCH):
                    t2 = t2pool.tile([128, rows], BF16)
                    nc.vector.tensor_scalar(
                        out=t2[:], in0=r8b[:],
                        scalar1=qu[:, 2 * jc:2 * jc + 1],
                        scalar2=qu[:, 2 * jc + 1:2 * jc + 2],
                        op0=OP.mult, op1=OP.max)
                    t3 = t3pool.tile([128, rows], BF16)
                    ms = m2sb[:, jc * rows:(jc + 1) * rows]
                    nc.vector.tensor_mul(t3[:, 0:DVE_SPLIT], t2[:, 0:DVE_SPLIT],
                                         ms[:, 0:DVE_SPLIT])
                    nc.gpsimd.tensor_mul(t3[:, DVE_SPLIT:rows],
                                         t2[:, DVE_SPLIT:rows],
                                         ms[:, DVE_SPLIT:rows])
                    for ic in range(rch):
                        nc.tensor.matmul(
                            out=po[ic][:],
                            lhsT=t3[:, ic * 128:(ic + 1) * 128],
                            rhs=wh2sb[:, jc * WC2:(jc + 1) * WC2],
                            start=(jc == 0), stop=(jc == NCH - 1))

                # raw accumulators out; normalize/elu/log_softmax on host
                pod = consts.tile([128, rch * WC2], F32)
                for ic in range(rch):
                    eng = (nc.vector, nc.scalar, nc.vector, nc.scalar)[ic]
                    if eng is nc.scalar:
                        nc.scalar.activation(
                            out=pod[:, ic * WC2:(ic + 1) * WC2],
                            in_=po[ic][:], func=AF.Copy)
                    else:
                        nc.vector.tensor_copy(
                            out=pod[:, ic * WC2:(ic + 1) * WC2],
                            in_=po[ic][:])
                nc.sync.dma_start(out=out[:, :], in_=pod[:])

    nc.compile()
    return nc


_CACHE = {}


def _get_programs():
    return _CACHE["p1"], _CACHE["p2"]


# --------------------------------------------------------------------------
# host-side prep
# --------------------------------------------------------------------------
def _sort_structure(f1, f2):
    """sigma (rows by f2), tau (cols by cutoff), cutoffs c, block bounds."""
    sigma = np.argsort(f2, kind="stable")
    f2s = f2[sigma]
    c = np.searchsorted(f2s, -f1, side="left")   # branch-1 count per col
    tau = np.argsort(c, kind="stable")
    cs = c[tau]
    b = cs // 128                                 # boundary chunk per col
    lo = np.minimum(b.reshape(NCH, 128).min(axis=1), NCH - 1)
    hi = np.minimum(b.reshape(NCH, 128).max(axis=1), NCH - 1)
    return sigma, tau, cs, lo, hi


def prep_phase1(x, adj, W_heads, a1_heads, a2_heads):
    maskT8 = (adj > 0).T.astype(NPF8)            # maskT[j, i] = adj[i, j]
    mu8 = maskT8.view(np.uint8)

    heads = []
    los = np.full(NCH, NCH - 1, np.int64)
    his = np.zeros(NCH, np.int64)
    for h in range(HEADS):
        Wh = (x @ W_heads[h]).astype(np.float32)          # [N, 64]
        f1 = Wh @ a1_heads[h]
        f2 = Wh @ a2_heads[h]
        sigma, tau, cs, lo, hi = _sort_structure(f1, f2)
        los = np.minimum(los, lo)
        his = np.maximum(his, hi)
        heads.append((Wh, f1, f2, sigma, tau, cs))
    spans = tuple((int(l), int(h)) for l, h in zip(los, his))
    sb = sum(h - l + 1 for l, h in spans)

    in1 = []
    for h in range(HEADS):
        Wh, f1, f2, sigma, tau, cs = heads[h]
        mp = mu8[np.ix_(sigma, tau)].copy()               # [N, N] permuted
        # boundary splits: bd1 ships separately; the branch-2 half
        # overwrites the boundary blocks of mp (consumed via the slab)
        bd1 = np.zeros((128, sb * 128), np.uint8)
        idx = 0
        for k, (lo, hi) in enumerate(spans):
            ck = cs[k * 128:(k + 1) * 128]                # cutoffs, this block
            for q in range(lo, hi + 1):
                mblk = mp[q * 128:(q + 1) * 128, k * 128:(k + 1) * 128]
                r = np.arange(q * 128, q * 128 + 128)[:, None]
                br1 = (r < ck[None, :])
                bd1[:, idx * 128:(idx + 1) * 128] = np.where(br1, mblk, 0)
                mp[q * 128:(q + 1) * 128,
                   k * 128:(k + 1) * 128] = np.where(br1, 0, mblk)
                idx += 1
        # slabs: [16][p][q][256] = mp[128q + p, 256r + c]
        mps = np.ascontiguousarray(
            mp.reshape(NCH, 128, 16, 256).transpose(2, 1, 0, 3)
            .reshape(16, 128, NCH * 256)).view(NPF8)
        f2s = f2[sigma]
        q2p = np.exp(ALPHA * f2s).astype(np.float32)      # e^{0.2 f2}
        u2p = np.exp(f2s).astype(np.float32)              # e^{f2}
        Whs = Wh[sigma]                                   # sorted rows
        rqf = np.concatenate([Whs * q2p[:, None], q2p[:, None],
                              np.zeros((N, 1), np.float32)], axis=1)
        ruf = np.concatenate([Whs * u2p[:, None], u2p[:, None],
                              np.zeros((N, 1), np.float32)], axis=1)
        # [p][q][f] layout
        rqd = np.ascontiguousarray(
            rqf.reshape(NCH, 128, WC1).transpose(1, 0, 2)
            .reshape(128, NCH * WC1)).astype(NPBF)
        rud = np.ascontiguousarray(
            ruf.reshape(NCH, 128, WC1).transpose(1, 0, 2)
            .reshape(128, NCH * WC1)).astype(NPBF)
        f1t = f1[tau]
        abf = np.stack([np.exp(ALPHA * f1t), np.exp(f1t)],
                       axis=1).astype(np.float32)         # [N, 2] A|B
        abd = np.ascontiguousarray(
            abf.reshape(NCH, 128, 2).transpose(1, 0, 2)
            .reshape(128, NCH * 2))
        in1.append({"mps": mps, "bd1": bd1.view(NPF8),
                    "rqd": rqd, "rud": rud, "abd": abd})
    return in1, heads, spans


def finish_phase1(r1, heads):
    """num/den -> h (elu'd, concatenated) in natural node order."""
    H = np.zeros((N, HEADS * HID), np.float32)
    for h in range(HEADS):
        tau = heads[h][4]
        numo = r1[h]["numo"].astype(np.float32)           # [128, NCH*WC1]
        ns = numo.reshape(128, NCH, WC1).transpose(1, 0, 2).reshape(N, WC1)
        hv = ns[:, 0:HID] / ns[:, HID:HID + 1]            # sorted cols
        hn = np.empty_like(hv)
        hn[tau] = hv                                      # un-permute
        H[:, h * HID:(h + 1) * HID] = np.where(hn > 0, hn, np.expm1(hn))
    return H


def prep_phase2(H, adj, W_out, a1_out, a2_out):
    maskT8 = (adj > 0).T.astype(NPF8)
    Wh2 = (H @ W_out).astype(np.float32)                  # [N, 256]
    f1o = Wh2 @ a1_out
    f2o = Wh2 @ a2_out
    wh2f = np.concatenate([Wh2, np.ones((N, 1), np.float32)], axis=1)
    wh2d = np.ascontiguousarray(
        wh2f.reshape(NCH, 128, WC2).transpose(1, 0, 2)
        .reshape(128, NCH * WC2)).astype(NPBF)
    quf = np.stack([np.exp(f2o), np.exp(ALPHA * f2o)],
                   axis=1).astype(np.float32)             # u2o | q2o
    qud = np.ascontiguousarray(
        quf.reshape(NCH, 128, 2).transpose(1, 0, 2).reshape(128, NCH * 2))
    in2 = []
    for c in range(NCORES):
        rs = slice(c * ROWS2, (c + 1) * ROWS2)
        m2 = np.ascontiguousarray(
            maskT8.reshape(NCH, 128, N)[:, :, rs]
            .transpose(1, 0, 2).reshape(128, NCH * ROWS2))
        r8 = np.exp(0.8 * f1o[rs]).astype(NPBF)[None, :]
        in2.append({"wh2": wh2d, "m2": m2, "r8d": r8, "qud": qud})
    return in2


def kernel(x, adj, W_heads, a1_heads, a2_heads, W_out, a1_out, a2_out, **_):
    x = np.asarray(x, dtype=np.float32)
    adj = np.asarray(adj)
    W_heads = np.asarray(W_heads, dtype=np.float32)
    a1_heads = np.asarray(a1_heads, dtype=np.float32)
    a2_heads = np.asarray(a2_heads, dtype=np.float32)
    W_out = np.asarray(W_out, dtype=np.float32)
    a1_out = np.asarray(a1_out, dtype=np.float32)
    a2_out = np.asarray(a2_out, dtype=np.float32)

    in1, heads, spans = prep_phase1(x, adj, W_heads, a1_heads, a2_heads)
    if _CACHE.get("spans") != spans:
        _CACHE["p1"] = build_phase1(spans)
        _CACHE["spans"] = spans
    if "p2" not in _CACHE:
        _CACHE["p2"] = build_phase2()
    p1, p2 = _CACHE["p1"], _CACHE["p2"]

    r1 = run_bass_kernel_spmd(p1, in1, core_ids=list(range(NCORES))).results
    H = finish_phase1(r1, heads)
    in2 = prep_phase2(H, adj, W_out, a1_out, a2_out)
    r2 = run_bass_kernel_spmd(p2, in2, core_ids=list(range(NCORES))).results
    # host epilogue: normalize, elu, log_softmax per core's raw accumulators
    outs = []
    for c in range(NCORES):
        po = np.asarray(r2[c]["out"], np.float32)         # [128, rch*WC2]
        po = po.reshape(128, ROWS2 // 128, WC2).transpose(1, 0, 2) \
               .reshape(ROWS2, WC2)
        an = po[:, 0:FOUT] / po[:, FOUT:FOUT + 1]
        el = np.where(an > 0, an, np.expm1(an))
        el -= np.log(np.exp(el).sum(axis=1, keepdims=True))
        outs.append(el)
    return np.concatenate(outs, axis=0).astype(np.float32)


# revision 20
# speedup vs baseline: 2.9682x; 1.3463x over previous
"""GAT (2-layer, 8-head) Trainium2 kernel, 8-core SPMD — sort-based layer 1.

Layer 1 (head-parallel, one head per core) avoids materializing the [N,N]
score matrix entirely.  With g = f1_i + f2_j and p = exp(leakyrelu(g))*m:
    p = A[i]*q2'[j]*m          when g < 0   (A = e^{0.2 f1},  q2' = e^{0.2 f2})
    p = B[i]*u2'[j]*m          when g >= 0  (B = e^{f1},      u2' = e^{f2})
Sorting j by f2 and i by the cutoff c(i) = #{j : f2_j < -f1_i} makes the
branch a block predicate: for column-block k (128 sorted i's) and row-chunk
q (128 sorted j's), q < b_lo(k) is pure branch-1, q > b_hi(k) pure branch-2,
and the few boundary chunks are host-presplit into bd1/bd2 = m*1[branch].
The PE then consumes the raw permuted mask (fp8, exact for 0/1) as the
stationary operand and streams [Wh*q2'|q2'] / [Wh*u2'|u2'] (66 bf16 rows)
as the moving operand, accumulating S1/S2 per block in PSUM; the drain
combines num = A*S1 + B*S2 on Act/DVE.  No elementwise pass ever touches
an [N,N] tensor, so the phase is paced by the 16.7MB fp8 mask DMA.

Layer 2 (row-parallel, 512 rows per core) keeps the dense-score pipeline but
with scores p~ = max(q2o[j], r8[i]*u2o[j]) * m  (r8 = e^{0.8 f1o}; the
e^{0.2 f1_i} factor cancels in the softmax): one fused 2-scalar tensor_scalar
(DVE 4x mode) plus one mask multiply split DVE/Pool, then the attention
matmul with plain [Wh2|1] as the moving operand.

The host does the O(N*F) prep (projections, exp vectors, sorts, mask
permutations and boundary splits, inter-layer elu/concat) in numpy; both
bass programs are built at first kernel() call from the inputs' cutoff
structure (uniform across cores so the programs stay SPMD).
"""

import sys

for p in ("/opt/trn_rl_repo", "/opt/pypackages"):
    if p not in sys.path:
        sys.path.append(p)

import numpy as np
import ml_dtypes

import concourse.bass as bass
import concourse.bacc as bacc
import concourse.tile as tile
from concourse import mybir
from concourse.bass_utils import run_bass_kernel_spmd

BF16 = mybir.dt.bfloat16
FP8 = mybir.dt.float8e4
F32 = mybir.dt.float32
OP = mybir.AluOpType
AF = mybir.ActivationFunctionType

NPBF = ml_dtypes.bfloat16
NPF8 = ml_dtypes.float8_e4m3fn

N, FIN, HID, HEADS, FOUT = 4096, 512, 64, 8, 256
NCORES = 8
NCH = N // 128          # 32 row chunks / column blocks
WC1 = HID + 2           # 64 features | den | pad
WC2 = FOUT + 1          # 256 features | den
ROWS2 = N // NCORES     # 512 output rows per core in layer 2
ALPHA = 0.2


# --------------------------------------------------------------------------
# phase 1: sort-based head attention.  spans = ((b_lo, b_hi), ...) per block,
# uniform across heads; bdoff[k] = offset of block k's boundary tiles.
# --------------------------------------------------------------------------
def build_phase1(spans):
    nc = bacc.Bacc("TRN2", target_bir_lowering=False, debug=False,
                   enable_asserts=False)
    sb = sum(hi - lo + 1 for lo, hi in spans)
    bdoff = []
    off = 0
    for lo, hi in spans:
        bdoff.append(off)
        off += hi - lo + 1

    # slab r holds mask columns [256r, 256r+256) for all 32 row chunks,
    # host-packed contiguous per partition: [p][q][c] with c in-block col.
    # Boundary-chunk blocks of the slab are pre-split by the host to the
    # branch-2 part (bd2); the branch-1 part ships separately as bd1.
    mps = nc.dram_tensor("mps", [16, 128, NCH * 256], FP8,
                         kind="ExternalInput")
    bd1 = nc.dram_tensor("bd1", [128, sb * 128], FP8, kind="ExternalInput")
    rqd = nc.dram_tensor("rqd", [128, NCH * WC1], BF16, kind="ExternalInput")
    rud = nc.dram_tensor("rud", [128, NCH * WC1], BF16, kind="ExternalInput")
    abd = nc.dram_tensor("abd", [128, NCH * 2], F32, kind="ExternalInput")
    numo = nc.dram_tensor("numo", [128, NCH * WC1], BF16,
                          kind="ExternalOutput")

    with tile.TileContext(nc) as tc:
        with tc.tile_pool(name="consts", bufs=1) as consts:
            rq = consts.tile([128, NCH * WC1], BF16)
            ru = consts.tile([128, NCH * WC1], BF16)
            ab = consts.tile([128, NCH * 2], F32)
            bd1s = consts.tile([128, sb * 128], FP8)
            stage = consts.tile([128, NCH * WC1], BF16)
            h1 = 8 * WC1
            nc.gpsimd.dma_start(out=rq[:, 0:h1], in_=rqd[:, 0:h1])
            nc.gpsimd.dma_start(out=ru[:, 0:h1], in_=rud[:, 0:h1])
            nc.gpsimd.dma_start(out=rq[:, h1:], in_=rqd[:, h1:])
            nc.gpsimd.dma_start(out=ru[:, h1:], in_=rud[:, h1:])
            nc.gpsimd.dma_start(out=ab[:], in_=abd[:, :])
            bq = (sb + 3) // 4 * 128
            for i in range(4):
                c0, c1 = i * bq, min((i + 1) * bq, sb * 128)
                if c0 < c1:
                    nc.scalar.dma_start(out=bd1s[:, c0:c1],
                                        in_=bd1[:, c0:c1])
            with (
                tc.tile_pool(name="slabs", bufs=4) as slabs,
                tc.tile_pool(name="ps1", bufs=4, space="PSUM") as ps1p,
                tc.tile_pool(name="ps2", bufs=4, space="PSUM") as ps2p,
                tc.tile_pool(name="ep", bufs=4) as ep,
            ):
                for r in range(16):          # 2 column blocks per round
                    slab = slabs.tile([128, NCH * 256], FP8, name="slab",
                                      tag="slab")
                    eng = nc.sync if r % 2 == 0 else nc.gpsimd
                    eng.dma_start(
                        out=slab[:],
                        in_=bass.AP(tensor=mps, offset=r * 128 * NCH * 256,
                                    ap=[[NCH * 256, 128], [1, NCH * 256]]))
                    ks = (2 * r, 2 * r + 1)
                    ps1 = {k: ps1p.tile([128, WC1], F32, name=f"ps1_{k}",
                                        tag="ps1") for k in ks}
                    ps2 = {k: ps2p.tile([128, WC1], F32, name=f"ps2_{k}",
                                        tag="ps2") for k in ks}
                    for q in range(NCH):
                        for k in ks:
                            lo, hi = spans[k]
                            col = q * 256 + (k % 2) * 128
                            rqs = rq[:, q * WC1:(q + 1) * WC1]
                            rus = ru[:, q * WC1:(q + 1) * WC1]
                            if q < lo:
                                nc.tensor.matmul(
                                    out=ps1[k][:],
                                    lhsT=slab[:, col:col + 128],
                                    rhs=rqs, start=(q == 0), stop=False)
                            elif q > hi:
                                nc.tensor.matmul(
                                    out=ps2[k][:],
                                    lhsT=slab[:, col:col + 128],
                                    rhs=rus, start=False, stop=(q == NCH - 1))
                            else:
                                i0 = (bdoff[k] + q - lo) * 128
                                nc.tensor.matmul(
                                    out=ps1[k][:],
                                    lhsT=bd1s[:, i0:i0 + 128],
                                    rhs=rqs, start=(q == 0), stop=(q == hi))
                                nc.tensor.matmul(
                                    out=ps2[k][:],
                                    lhsT=slab[:, col:col + 128],
                                    rhs=rus, start=(q == lo),
                                    stop=(q == NCH - 1))
                    for k in ks:
                        t1 = ep.tile([128, WC1], BF16, name="t1", tag="t1")
                        nc.scalar.activation(out=t1[:], in_=ps1[k][:],
                                             func=AF.Copy,
                                             scale=ab[:, 2 * k:2 * k + 1])
                        nc.vector.scalar_tensor_tensor(
                            out=stage[:, k * WC1:(k + 1) * WC1],
                            in0=ps2[k][:], scalar=ab[:, 2 * k + 1:2 * k + 2],
                            in1=t1[:], op0=OP.mult, op1=OP.add)
                    if r == 7:
                        nc.sync.dma_start(out=numo[:, 0:16 * WC1],
                                          in_=stage[:, 0:16 * WC1])
            nc.sync.dma_start(out=numo[:, 16 * WC1:], in_=stage[:, 16 * WC1:])

    nc.compile()
    return nc


# --------------------------------------------------------------------------
# phase 2: dense-score layer-2 attention for 512 rows per core.
# --------------------------------------------------------------------------
def build_phase2():
    nc = bacc.Bacc("TRN2", target_bir_lowering=False, debug=False,
                   enable_asserts=False)
    rows = ROWS2
    rch = rows // 128
    AC = 128             # cols on the Act relu path (+ DVE fused add*mask)
    XS = 352             # end of the DVE tensor-mul range; rest on Pool

    wh2 = nc.dram_tensor("wh2", [128, NCH * WC2], BF16, kind="ExternalInput")
    m2 = nc.dram_tensor("m2", [128, NCH * rows], BF16, kind="ExternalInput")
    r8d = nc.dram_tensor("r8d", [1, rows], BF16, kind="ExternalInput")
    qud = nc.dram_tensor("qud", [128, NCH * 3], F32, kind="ExternalInput")
    out = nc.dram_tensor("out", [128, rch * WC2], F32, kind="ExternalOutput")

    with tile.TileContext(nc) as tc:
        with tc.tile_pool(name="consts", bufs=1) as consts:
            qu = consts.tile([128, NCH * 3], F32)
            nc.sync.dma_start(out=qu[:], in_=qud[:, :])
            r8b = consts.tile([128, rows], BF16)
            nc.sync.dma_start(
                out=r8b[:],
                in_=bass.AP(tensor=r8d, offset=0, ap=[[0, 128], [1, rows]]))
            wh2sb = consts.tile([128, NCH * WC2], BF16)
            m2sb = consts.tile([128, NCH * rows], BF16)
            for i in range(8):
                nc.scalar.dma_start(
                    out=m2sb[:, i * 4 * rows:(i + 1) * 4 * rows],
                    in_=m2[:, i * 4 * rows:(i + 1) * 4 * rows])
                nc.gpsimd.dma_start(
                    out=wh2sb[:, i * 4 * WC2:(i + 1) * 4 * WC2],
                    in_=wh2[:, i * 4 * WC2:(i + 1) * 4 * WC2])

            with (
                tc.tile_pool(name="t2pool", bufs=6) as t2pool,
                tc.tile_pool(name="t3pool", bufs=6) as t3pool,
                tc.tile_pool(name="ps4", bufs=rch, space="PSUM") as ps4,
            ):
                po = [ps4.tile([128, WC2], F32, name=f"po{_i}", tag="po")
                      for _i in range(rch)]
                for jc in range(NCH):
                    ms = m2sb[:, jc * rows:(jc + 1) * rows]
                    t2 = t2pool.tile([128, rows], BF16)
                    nc.vector.tensor_scalar(
                        out=t2[:], in0=r8b[:],
                        scalar1=qu[:, 3 * jc:3 * jc + 1],
                        scalar2=qu[:, 3 * jc + 1:3 * jc + 2],
                        op0=OP.mult, op1=OP.max)
                    t3 = t3pool.tile([128, rows], BF16)
                    nc.vector.tensor_mul(t3[:, 0:XS], t2[:, 0:XS],
                                         ms[:, 0:XS])
                    nc.gpsimd.tensor_mul(t3[:, XS:rows], t2[:, XS:],
                                         ms[:, XS:rows])
                    for ic in range(rch):
                        nc.tensor.matmul(
                            out=po[ic][:],
                            lhsT=t3[:, ic * 128:(ic + 1) * 128],
                            rhs=wh2sb[:, jc * WC2:(jc + 1) * WC2],
                            start=(jc == 0), stop=(jc == NCH - 1))

                # raw accumulators out; normalize/elu/log_softmax on host
                pod = consts.tile([128, rch * WC2], F32)
                for ic in range(rch):
                    eng = (nc.vector, nc.scalar, nc.vector, nc.scalar)[ic]
                    if eng is nc.scalar:
                        nc.scalar.activation(
                            out=pod[:, ic * WC2:(ic + 1) * WC2],
                            in_=po[ic][:], func=AF.Copy)
                    else:
                        nc.vector.tensor_copy(
                            out=pod[:, ic * WC2:(ic + 1) * WC2],
                            in_=po[ic][:])
                nc.sync.dma_start(out=out[:, :], in_=pod[:])

    nc.compile()
    return nc


_CACHE = {}


def _get_programs():
    return _CACHE["p1"], _CACHE["p2"]


# --------------------------------------------------------------------------
# host-side prep
# --------------------------------------------------------------------------
def _sort_structure(f1, f2):
    """sigma (rows by f2), tau (cols by cutoff), cutoffs c, block bounds."""
    sigma = np.argsort(f2, kind="stable")
    f2s = f2[sigma]
    c = np.searchsorted(f2s, -f1, side="left")   # branch-1 count per col
    tau = np.argsort(c, kind="stable")
    cs = c[tau]
    b = cs // 128                                 # boundary chunk per col
    lo = np.minimum(b.reshape(NCH, 128).min(axis=1), NCH - 1)
    hi = np.minimum(b.reshape(NCH, 128).max(axis=1), NCH - 1)
    return sigma, tau, cs, lo, hi


def prep_phase1(x, adj, W_heads, a1_heads, a2_heads):
    maskT8 = (adj > 0).T.astype(NPF8)            # maskT[j, i] = adj[i, j]
    mu8 = maskT8.view(np.uint8)

    heads = []
    los = np.full(NCH, NCH - 1, np.int64)
    his = np.zeros(NCH, np.int64)
    for h in range(HEADS):
        Wh = (x @ W_heads[h]).astype(np.float32)          # [N, 64]
        f1 = Wh @ a1_heads[h]
        f2 = Wh @ a2_heads[h]
        sigma, tau, cs, lo, hi = _sort_structure(f1, f2)
        los = np.minimum(los, lo)
        his = np.maximum(his, hi)
        heads.append((Wh, f1, f2, sigma, tau, cs))
    spans = tuple((int(l), int(h)) for l, h in zip(los, his))
    sb = sum(h - l + 1 for l, h in spans)

    in1 = []
    for h in range(HEADS):
        Wh, f1, f2, sigma, tau, cs = heads[h]
        mp = mu8[np.ix_(sigma, tau)].copy()               # [N, N] permuted
        # boundary splits: bd1 ships separately; the branch-2 half
        # overwrites the boundary blocks of mp (consumed via the slab)
        bd1 = np.zeros((128, sb * 128), np.uint8)
        idx = 0
        for k, (lo, hi) in enumerate(spans):
            ck = cs[k * 128:(k + 1) * 128]                # cutoffs, this block
            for q in range(lo, hi + 1):
                mblk = mp[q * 128:(q + 1) * 128, k * 128:(k + 1) * 128]
                r = np.arange(q * 128, q * 128 + 128)[:, None]
                br1 = (r < ck[None, :])
                bd1[:, idx * 128:(idx + 1) * 128] = np.where(br1, mblk, 0)
                mp[q * 128:(q + 1) * 128,
                   k * 128:(k + 1) * 128] = np.where(br1, 0, mblk)
                idx += 1
        # slabs: [16][p][q][256] = mp[128q + p, 256r + c]
        mps = np.ascontiguousarray(
            mp.reshape(NCH, 128, 16, 256).transpose(2, 1, 0, 3)
            .reshape(16, 128, NCH * 256)).view(NPF8)
        f2s = f2[sigma]
        q2p = np.exp(ALPHA * f2s).astype(np.float32)      # e^{0.2 f2}
        u2p = np.exp(f2s).astype(np.float32)              # e^{f2}
        Whs = Wh[sigma]                                   # sorted rows
        rqf = np.concatenate([Whs * q2p[:, None], q2p[:, None],
                              np.zeros((N, 1), np.float32)], axis=1)
        ruf = np.concatenate([Whs * u2p[:, None], u2p[:, None],
                              np.zeros((N, 1), np.float32)], axis=1)
        # [p][q][f] layout
        rqd = np.ascontiguousarray(
            rqf.reshape(NCH, 128, WC1).transpose(1, 0, 2)
            .reshape(128, NCH * WC1)).astype(NPBF)
        rud = np.ascontiguousarray(
            ruf.reshape(NCH, 128, WC1).transpose(1, 0, 2)
            .reshape(128, NCH * WC1)).astype(NPBF)
        f1t = f1[tau]
        abf = np.stack([np.exp(ALPHA * f1t), np.exp(f1t)],
                       axis=1).astype(np.float32)         # [N, 2] A|B
        abd = np.ascontiguousarray(
            abf.reshape(NCH, 128, 2).transpose(1, 0, 2)
            .reshape(128, NCH * 2))
        in1.append({"mps": mps, "bd1": bd1.view(NPF8),
                    "rqd": rqd, "rud": rud, "abd": abd})
    return in1, heads, spans


def finish_phase1(r1, heads):
    """num/den -> h (elu'd, concatenated) in natural node order."""
    H = np.zeros((N, HEADS * HID), np.float32)
    for h in range(HEADS):
        tau = heads[h][4]
        numo = r1[h]["numo"].astype(np.float32)           # [128, NCH*WC1]
        ns = numo.reshape(128, NCH, WC1).transpose(1, 0, 2).reshape(N, WC1)
        hv = ns[:, 0:HID] / ns[:, HID:HID + 1]            # sorted cols
        hn = np.empty_like(hv)
        hn[tau] = hv                                      # un-permute
        H[:, h * HID:(h + 1) * HID] = np.where(hn > 0, hn, np.expm1(hn))
    return H


def prep_phase2(H, adj, W_out, a1_out, a2_out):
    maskT8 = (adj > 0).T.astype(NPBF)
    Wh2 = (H @ W_out).astype(np.float32)                  # [N, 256]
    f1o = Wh2 @ a1_out
    f2o = Wh2 @ a2_out
    wh2f = np.concatenate([Wh2, np.ones((N, 1), np.float32)], axis=1)
    wh2d = np.ascontiguousarray(
        wh2f.reshape(NCH, 128, WC2).transpose(1, 0, 2)
        .reshape(128, NCH * WC2)).astype(NPBF)
    q2o = np.exp(ALPHA * f2o)
    quf = np.stack([np.exp(f2o), q2o, -q2o],
                   axis=1).astype(np.float32)             # u2o | q2o | -q2o
    qud = np.ascontiguousarray(
        quf.reshape(NCH, 128, 3).transpose(1, 0, 2).reshape(128, NCH * 3))
    in2 = []
    for c in range(NCORES):
        rs = slice(c * ROWS2, (c + 1) * ROWS2)
        m2 = np.ascontiguousarray(
            maskT8.reshape(NCH, 128, N)[:, :, rs]
            .transpose(1, 0, 2).reshape(128, NCH * ROWS2))
        r8 = np.exp(0.8 * f1o[rs]).astype(NPBF)[None, :]
        in2.append({"wh2": wh2d, "m2": m2, "r8d": r8, "qud": qud})
    return in2


def kernel(x, adj, W_heads, a1_heads, a2_heads, W_out, a1_out, a2_out, **_):
    x = np.asarray(x, dtype=np.float32)
    adj = np.asarray(adj)
    W_heads = np.asarray(W_heads, dtype=np.float32)
    a1_heads = np.asarray(a1_heads, dtype=np.float32)
    a2_heads = np.asarray(a2_heads, dtype=np.float32)
    W_out = np.asarray(W_out, dtype=np.float32)
    a1_out = np.asarray(a1_out, dtype=np.float32)
    a2_out = np.asarray(a2_out, dtype=np.float32)

    in1, heads, spans = prep_phase1(x, adj, W_heads, a1_heads, a2_heads)
    if _CACHE.get("spans") != spans:
        _CACHE["p1"] = build_phase1(spans)
        _CACHE["spans"] = spans
    if "p2" not in _CACHE:
        _CACHE["p2"] = build_phase2()
    p1, p2 = _CACHE["p1"], _CACHE["p2"]

    r1 = run_bass_kernel_spmd(p1, in1, core_ids=list(range(NCORES))).results
    H = finish_phase1(r1, heads)
    in2 = prep_phase2(H, adj, W_out, a1_out, a2_out)
    r2 = run_bass_kernel_spmd(p2, in2, core_ids=list(range(NCORES))).results
    # host epilogue: normalize, elu, log_softmax per core's raw accumulators
    outs = []
    for c in range(NCORES):
        po = np.asarray(r2[c]["out"], np.float32)         # [128, rch*WC2]
        po = po.reshape(128, ROWS2 // 128, WC2).transpose(1, 0, 2) \
               .reshape(ROWS2, WC2)
        an = po[:, 0:FOUT] / po[:, FOUT:FOUT + 1]
        el = np.where(an > 0, an, np.expm1(an))
        el -= np.log(np.exp(el).sum(axis=1, keepdims=True))
        outs.append(el)
    return np.concatenate(outs, axis=0).astype(np.float32)


# revision 24
# speedup vs baseline: 3.0540x; 1.0289x over previous
"""GAT (2-layer, 8-head) Trainium2 kernel, 8-core SPMD — sort-based layer 1.

Layer 1 (head-parallel, one head per core) avoids materializing the [N,N]
score matrix entirely.  With g = f1_i + f2_j and p = exp(leakyrelu(g))*m:
    p = A[i]*q2'[j]*m          when g < 0   (A = e^{0.2 f1},  q2' = e^{0.2 f2})
    p = B[i]*u2'[j]*m          when g >= 0  (B = e^{f1},      u2' = e^{f2})
Sorting j by f2 and i by the cutoff c(i) = #{j : f2_j < -f1_i} makes the
branch a block predicate: for column-block k (128 sorted i's) and row-chunk
q (128 sorted j's), q < b_lo(k) is pure branch-1, q > b_hi(k) pure branch-2,
and the few boundary chunks are host-presplit into bd1/bd2 = m*1[branch].
The PE then consumes the raw permuted mask (fp8, exact for 0/1) as the
stationary operand and streams [Wh*q2'|q2'] / [Wh*u2'|u2'] (66 bf16 rows)
as the moving operand, accumulating S1/S2 per block in PSUM; the drain
combines num = A*S1 + B*S2 on Act/DVE.  No elementwise pass ever touches
an [N,N] tensor, so the phase is paced by the 16.7MB fp8 mask DMA.

Layer 2 (row-parallel, 512 rows per core) keeps the dense-score pipeline but
with scores p~ = max(q2o[j], r8[i]*u2o[j]) * m  (r8 = e^{0.8 f1o}; the
e^{0.2 f1_i} factor cancels in the softmax): one fused 2-scalar tensor_scalar
(DVE 4x mode) plus one mask multiply split DVE/Pool, then the attention
matmul with plain [Wh2|1] as the moving operand.

The host does the O(N*F) prep (projections, exp vectors, sorts, mask
permutations and boundary splits, inter-layer elu/concat) in numpy; both
bass programs are built at first kernel() call from the inputs' cutoff
structure (uniform across cores so the programs stay SPMD).
"""

import sys

for p in ("/opt/trn_rl_repo", "/opt/pypackages"):
    if p not in sys.path:
        sys.path.append(p)

import numpy as np
import ml_dtypes

import concourse.bass as bass
import concourse.bacc as bacc
import concourse.tile as tile
from concourse import mybir
from concourse.bass_utils import run_bass_kernel_spmd

BF16 = mybir.dt.bfloat16
FP8 = mybir.dt.float8e4
F32 = mybir.dt.float32
OP = mybir.AluOpType
AF = mybir.ActivationFunctionType

NPBF = ml_dtypes.bfloat16
NPF8 = ml_dtypes.float8_e4m3fn

N, FIN, HID, HEADS, FOUT = 4096, 512, 64, 8, 256
NCORES = 8
NCH = N // 128          # 32 row chunks / column blocks
WC1 = HID + 2           # 64 features | den | pad
WC2 = FOUT + 1          # 256 features | den
ROWS2 = N // NCORES     # 512 output rows per core in layer 2
ALPHA = 0.2


# --------------------------------------------------------------------------
# phase 1: sort-based head attention.  spans = ((b_lo, b_hi), ...) per block,
# uniform across heads; bdoff[k] = offset of block k's boundary tiles.
# --------------------------------------------------------------------------
def build_phase1(spans):
    nc = bacc.Bacc("TRN2", target_bir_lowering=False, debug=False,
                   enable_asserts=False)
    sb = sum(hi - lo + 1 for lo, hi in spans)
    bdoff = []
    off = 0
    for lo, hi in spans:
        bdoff.append(off)
        off += hi - lo + 1

    # slab r holds mask columns [256r, 256r+256) for all 32 row chunks,
    # host-packed contiguous per partition: [p][q][c] with c in-block col.
    # Boundary-chunk blocks of the slab are pre-split by the host to the
    # branch-2 part (bd2); the branch-1 part ships separately as bd1.
    mps = nc.dram_tensor("mps", [16, 128, NCH * 256], FP8,
                         kind="ExternalInput")
    bd1 = nc.dram_tensor("bd1", [128, sb * 128], FP8, kind="ExternalInput")
    rqd = nc.dram_tensor("rqd", [128, NCH * WC1], BF16, kind="ExternalInput")
    rud = nc.dram_tensor("rud", [128, NCH * WC1], BF16, kind="ExternalInput")
    abd = nc.dram_tensor("abd", [128, NCH * 2], F32, kind="ExternalInput")
    numo = nc.dram_tensor("numo", [128, NCH * WC1], BF16,
                          kind="ExternalOutput")

    with tile.TileContext(nc) as tc:
        with tc.tile_pool(name="consts", bufs=1) as consts:
            rq = consts.tile([128, NCH * WC1], BF16)
            ru = consts.tile([128, NCH * WC1], BF16)
            ab = consts.tile([128, NCH * 2], F32)
            bd1s = consts.tile([128, sb * 128], FP8)
            stage = consts.tile([128, NCH * WC1], BF16)
            h1 = 8 * WC1
            nc.gpsimd.dma_start(out=rq[:, 0:h1], in_=rqd[:, 0:h1])
            nc.gpsimd.dma_start(out=ru[:, 0:h1], in_=rud[:, 0:h1])
            nc.gpsimd.dma_start(out=rq[:, h1:], in_=rqd[:, h1:])
            nc.gpsimd.dma_start(out=ru[:, h1:], in_=rud[:, h1:])
            nc.gpsimd.dma_start(out=ab[:], in_=abd[:, :])
            bq = (sb + 3) // 4 * 128
            for i in range(4):
                c0, c1 = i * bq, min((i + 1) * bq, sb * 128)
                if c0 < c1:
                    nc.scalar.dma_start(out=bd1s[:, c0:c1],
                                        in_=bd1[:, c0:c1])
            with (
                tc.tile_pool(name="slabs", bufs=4) as slabs,
                tc.tile_pool(name="ps1", bufs=4, space="PSUM") as ps1p,
                tc.tile_pool(name="ps2", bufs=4, space="PSUM") as ps2p,
                tc.tile_pool(name="ep", bufs=4) as ep,
            ):
                for r in range(16):          # 2 column blocks per round
                    slab = slabs.tile([128, NCH * 256], FP8, name="slab",
                                      tag="slab")
                    eng = nc.sync if r % 2 == 0 else nc.gpsimd
                    hw = NCH * 128
                    for hf in range(2):
                        eng.dma_start(
                            out=slab[:, hf * hw:(hf + 1) * hw],
                            in_=bass.AP(
                                tensor=mps,
                                offset=r * 128 * NCH * 256 + hf * hw,
                                ap=[[NCH * 256, 128], [1, hw]]))
                    ks = (2 * r, 2 * r + 1)
                    ps1 = {k: ps1p.tile([128, WC1], F32, name=f"ps1_{k}",
                                        tag="ps1") for k in ks}
                    ps2 = {k: ps2p.tile([128, WC1], F32, name=f"ps2_{k}",
                                        tag="ps2") for k in ks}
                    for q in range(NCH):
                        for k in ks:
                            lo, hi = spans[k]
                            col = q * 256 + (k % 2) * 128
                            rqs = rq[:, q * WC1:(q + 1) * WC1]
                            rus = ru[:, q * WC1:(q + 1) * WC1]
                            if q < lo:
                                nc.tensor.matmul(
                                    out=ps1[k][:],
                                    lhsT=slab[:, col:col + 128],
                                    rhs=rqs, start=(q == 0), stop=False)
                            elif q > hi:
                                nc.tensor.matmul(
                                    out=ps2[k][:],
                                    lhsT=slab[:, col:col + 128],
                                    rhs=rus, start=False, stop=(q == NCH - 1))
                            else:
                                i0 = (bdoff[k] + q - lo) * 128
                                nc.tensor.matmul(
                                    out=ps1[k][:],
                                    lhsT=bd1s[:, i0:i0 + 128],
                                    rhs=rqs, start=(q == 0), stop=(q == hi))
                                nc.tensor.matmul(
                                    out=ps2[k][:],
                                    lhsT=slab[:, col:col + 128],
                                    rhs=rus, start=(q == lo),
                                    stop=(q == NCH - 1))
                    for k in ks:
                        t1 = ep.tile([128, WC1], BF16, name="t1", tag="t1")
                        nc.scalar.activation(out=t1[:], in_=ps1[k][:],
                                             func=AF.Copy,
                                             scale=ab[:, 2 * k:2 * k + 1])
                        nc.vector.scalar_tensor_tensor(
                            out=stage[:, k * WC1:(k + 1) * WC1],
                            in0=ps2[k][:], scalar=ab[:, 2 * k + 1:2 * k + 2],
                            in1=t1[:], op0=OP.mult, op1=OP.add)
                    c0, c1 = 2 * r * WC1, (2 * r + 2) * WC1
                    nc.scalar.dma_start(out=numo[:, c0:c1],
                                        in_=stage[:, c0:c1])

    nc.compile()
    return nc


# --------------------------------------------------------------------------
# phase 2: dense-score layer-2 attention for 512 rows per core.
# --------------------------------------------------------------------------
def build_phase2():
    nc = bacc.Bacc("TRN2", target_bir_lowering=False, debug=False,
                   enable_asserts=False)
    rows = ROWS2
    rch = rows // 128
    AC = 128             # cols on the Act relu path (+ DVE fused add*mask)
    XS = 352             # end of the DVE tensor-mul range; rest on Pool

    wh2 = nc.dram_tensor("wh2", [128, NCH * WC2], BF16, kind="ExternalInput")
    m2 = nc.dram_tensor("m2", [128, NCH * rows], BF16, kind="ExternalInput")
    r8d = nc.dram_tensor("r8d", [1, rows], BF16, kind="ExternalInput")
    qud = nc.dram_tensor("qud", [128, NCH * 3], F32, kind="ExternalInput")
    out = nc.dram_tensor("out", [128, rch * WC2], F32, kind="ExternalOutput")

    with tile.TileContext(nc) as tc:
        with tc.tile_pool(name="consts", bufs=1) as consts:
            qu = consts.tile([128, NCH * 3], F32)
            nc.sync.dma_start(out=qu[:], in_=qud[:, :])
            r8b = consts.tile([128, rows], BF16)
            nc.sync.dma_start(
                out=r8b[:],
                in_=bass.AP(tensor=r8d, offset=0, ap=[[0, 128], [1, rows]]))
            wh2sb = consts.tile([128, NCH * WC2], BF16)
            m2sb = consts.tile([128, NCH * rows], BF16)
            for i in range(16):
                nc.scalar.dma_start(
                    out=m2sb[:, i * 2 * rows:(i + 1) * 2 * rows],
                    in_=m2[:, i * 2 * rows:(i + 1) * 2 * rows])
                if i < 8:
                    nc.gpsimd.dma_start(
                        out=wh2sb[:, i * 4 * WC2:(i + 1) * 4 * WC2],
                        in_=wh2[:, i * 4 * WC2:(i + 1) * 4 * WC2])

            with (
                tc.tile_pool(name="t2pool", bufs=6) as t2pool,
                tc.tile_pool(name="t3pool", bufs=6) as t3pool,
                tc.tile_pool(name="ps4", bufs=rch, space="PSUM") as ps4,
            ):
                po = [ps4.tile([128, WC2], F32, name=f"po{_i}", tag="po")
                      for _i in range(rch)]
                for jc in range(NCH):
                    ms = m2sb[:, jc * rows:(jc + 1) * rows]
                    t2 = t2pool.tile([128, rows], BF16)
                    nc.vector.tensor_scalar(
                        out=t2[:], in0=r8b[:],
                        scalar1=qu[:, 3 * jc:3 * jc + 1],
                        scalar2=qu[:, 3 * jc + 1:3 * jc + 2],
                        op0=OP.mult, op1=OP.max)
                    t3 = t3pool.tile([128, rows], BF16)
                    nc.vector.tensor_mul(t3[:, 0:XS], t2[:, 0:XS],
                                         ms[:, 0:XS])
                    nc.gpsimd.tensor_mul(t3[:, XS:rows], t2[:, XS:],
                                         ms[:, XS:rows])
                    for ic in range(rch):
                        nc.tensor.matmul(
                            out=po[ic][:],
                            lhsT=t3[:, ic * 128:(ic + 1) * 128],
                            rhs=wh2sb[:, jc * WC2:(jc + 1) * WC2],
                            start=(jc == 0), stop=(jc == NCH - 1))

                # raw accumulators out; normalize/elu/log_softmax on host
                pod = consts.tile([128, rch * WC2], F32)
                for ic in range(rch):
                    osl = slice(ic * WC2, (ic + 1) * WC2)
                    if ic % 2 == 0:
                        nc.vector.tensor_copy(out=pod[:, osl], in_=po[ic][:])
                    else:
                        nc.scalar.activation(out=pod[:, osl], in_=po[ic][:],
                                             func=AF.Copy)
                    deng = (nc.sync, nc.scalar, nc.sync, nc.scalar)[ic]
                    deng.dma_start(out=out[:, osl], in_=pod[:, osl])

    nc.compile()
    return nc


_CACHE = {}


def _get_programs():
    return _CACHE["p1"], _CACHE["p2"]


# --------------------------------------------------------------------------
# host-side prep
# --------------------------------------------------------------------------
def _sort_structure(f1, f2):
    """sigma (rows by f2), tau (cols by cutoff), cutoffs c, block bounds."""
    sigma = np.argsort(f2, kind="stable")
    f2s = f2[sigma]
    c = np.searchsorted(f2s, -f1, side="left")   # branch-1 count per col
    tau = np.argsort(c, kind="stable")
    cs = c[tau]
    b = cs // 128                                 # boundary chunk per col
    lo = np.minimum(b.reshape(NCH, 128).min(axis=1), NCH - 1)
    hi = np.minimum(b.reshape(NCH, 128).max(axis=1), NCH - 1)
    return sigma, tau, cs, lo, hi


def prep_phase1(x, adj, W_heads, a1_heads, a2_heads):
    maskT8 = (adj > 0).T.astype(NPF8)            # maskT[j, i] = adj[i, j]
    mu8 = maskT8.view(np.uint8)

    heads = []
    los = np.full(NCH, NCH - 1, np.int64)
    his = np.zeros(NCH, np.int64)
    for h in range(HEADS):
        Wh = (x @ W_heads[h]).astype(np.float32)          # [N, 64]
        f1 = Wh @ a1_heads[h]
        f2 = Wh @ a2_heads[h]
        sigma, tau, cs, lo, hi = _sort_structure(f1, f2)
        los = np.minimum(los, lo)
        his = np.maximum(his, hi)
        heads.append((Wh, f1, f2, sigma, tau, cs))
    spans = tuple((int(l), int(h)) for l, h in zip(los, his))
    sb = sum(h - l + 1 for l, h in spans)

    in1 = []
    for h in range(HEADS):
        Wh, f1, f2, sigma, tau, cs = heads[h]
        mp = mu8[np.ix_(sigma, tau)].copy()               # [N, N] permuted
        # boundary splits: bd1 ships separately; the branch-2 half
        # overwrites the boundary blocks of mp (consumed via the slab)
        bd1 = np.zeros((128, sb * 128), np.uint8)
        idx = 0
        for k, (lo, hi) in enumerate(spans):
            ck = cs[k * 128:(k + 1) * 128]                # cutoffs, this block
            for q in range(lo, hi + 1):
                mblk = mp[q * 128:(q + 1) * 128, k * 128:(k + 1) * 128]
                r = np.arange(q * 128, q * 128 + 128)[:, None]
                br1 = (r < ck[None, :])
                bd1[:, idx * 128:(idx + 1) * 128] = np.where(br1, mblk, 0)
                mp[q * 128:(q + 1) * 128,
                   k * 128:(k + 1) * 128] = np.where(br1, 0, mblk)
                idx += 1
        # slabs: [16][p][q][256] = mp[128q + p, 256r + c]
        mps = np.ascontiguousarray(
            mp.reshape(NCH, 128, 16, 256).transpose(2, 1, 0, 3)
            .reshape(16, 128, NCH * 256)).view(NPF8)
        f2s = f2[sigma]
        q2p = np.exp(ALPHA * f2s).astype(np.float32)      # e^{0.2 f2}
        u2p = np.exp(f2s).astype(np.float32)              # e^{f2}
        Whs = Wh[sigma]                                   # sorted rows
        rqf = np.concatenate([Whs * q2p[:, None], q2p[:, None],
                              np.zeros((N, 1), np.float32)], axis=1)
        ruf = np.concatenate([Whs * u2p[:, None], u2p[:, None],
                              np.zeros((N, 1), np.float32)], axis=1)
        # [p][q][f] layout
        rqd = np.ascontiguousarray(
            rqf.reshape(NCH, 128, WC1).transpose(1, 0, 2)
            .reshape(128, NCH * WC1)).astype(NPBF)
        rud = np.ascontiguousarray(
            ruf.reshape(NCH, 128, WC1).transpose(1, 0, 2)
            .reshape(128, NCH * WC1)).astype(NPBF)
        f1t = f1[tau]
        abf = np.stack([np.exp(ALPHA * f1t), np.exp(f1t)],
                       axis=1).astype(np.float32)         # [N, 2] A|B
        abd = np.ascontiguousarray(
            abf.reshape(NCH, 128, 2).transpose(1, 0, 2)
            .reshape(128, NCH * 2))
        in1.append({"mps": mps, "bd1": bd1.view(NPF8),
                    "rqd": rqd, "rud": rud, "abd": abd})
    return in1, heads, spans


def finish_phase1(r1, heads):
    """num/den -> h (elu'd, concatenated) in natural node order."""
    H = np.zeros((N, HEADS * HID), np.float32)
    for h in range(HEADS):
        tau = heads[h][4]
        numo = r1[h]["numo"].astype(np.float32)           # [128, NCH*WC1]
        ns = numo.reshape(128, NCH, WC1).transpose(1, 0, 2).reshape(N, WC1)
        hv = ns[:, 0:HID] / ns[:, HID:HID + 1]            # sorted cols
        hn = np.empty_like(hv)
        hn[tau] = hv                                      # un-permute
        H[:, h * HID:(h + 1) * HID] = np.where(hn > 0, hn, np.expm1(hn))
    return H


def prep_phase2(H, adj, W_out, a1_out, a2_out):
    maskT8 = (adj > 0).T.astype(NPBF)
    Wh2 = (H @ W_out).astype(np.float32)                  # [N, 256]
    f1o = Wh2 @ a1_out
    f2o = Wh2 @ a2_out
    wh2f = np.concatenate([Wh2, np.ones((N, 1), np.float32)], axis=1)
    wh2d = np.ascontiguousarray(
        wh2f.reshape(NCH, 128, WC2).transpose(1, 0, 2)
        .reshape(128, NCH * WC2)).astype(NPBF)
    q2o = np.exp(ALPHA * f2o)
    quf = np.stack([np.exp(f2o), q2o, -q2o],
                   axis=1).astype(np.float32)             # u2o | q2o | -q2o
    qud = np.ascontiguousarray(
        quf.reshape(NCH, 128, 3).transpose(1, 0, 2).reshape(128, NCH * 3))
    in2 = []
    for c in range(NCORES):
        rs = slice(c * ROWS2, (c + 1) * ROWS2)
        m2 = np.ascontiguousarray(
            maskT8.reshape(NCH, 128, N)[:, :, rs]
            .transpose(1, 0, 2).reshape(128, NCH * ROWS2))
        r8 = np.exp(0.8 * f1o[rs]).astype(NPBF)[None, :]
        in2.append({"wh2": wh2d, "m2": m2, "r8d": r8, "qud": qud})
    return in2


def kernel(x, adj, W_heads, a1_heads, a2_heads, W_out, a1_out, a2_out, **_):
    x = np.asarray(x, dtype=np.float32)
    adj = np.asarray(adj)
    W_heads = np.asarray(W_heads, dtype=np.float32)
    a1_heads = np.asarray(a1_heads, dtype=np.float32)
    a2_heads = np.asarray(a2_heads, dtype=np.float32)
    W_out = np.asarray(W_out, dtype=np.float32)
    a1_out = np.asarray(a1_out, dtype=np.float32)
    a2_out = np.asarray(a2_out, dtype=np.float32)

    in1, heads, spans = prep_phase1(x, adj, W_heads, a1_heads, a2_heads)
    if _CACHE.get("spans") != spans:
        _CACHE["p1"] = build_phase1(spans)
        _CACHE["spans"] = spans
    if "p2" not in _CACHE:
        _CACHE["p2"] = build_phase2()
    p1, p2 = _CACHE["p1"], _CACHE["p2"]

    r1 = run_bass_kernel_spmd(p1, in1, core_ids=list(range(NCORES))).results
    H = finish_phase1(r1, heads)
    in2 = prep_phase2(H, adj, W_out, a1_out, a2_out)
    r2 = run_bass_kernel_spmd(p2, in2, core_ids=list(range(NCORES))).results
    # host epilogue: normalize, elu, log_softmax per core's raw accumulators
    outs = []
    for c in range(NCORES):
        po = np.asarray(r2[c]["out"], np.float32)         # [128, rch*WC2]
        po = po.reshape(128, ROWS2 // 128, WC2).transpose(1, 0, 2) \
               .reshape(ROWS2, WC2)
        an = po[:, 0:FOUT] / po[:, FOUT:FOUT + 1]
        el = np.where(an > 0, an, np.expm1(an))
        el -= np.log(np.exp(el).sum(axis=1, keepdims=True))
        outs.append(el)
    return np.concatenate(outs, axis=0).astype(np.float32)


# revision 29
# speedup vs baseline: 3.0567x; 1.0009x over previous
"""GAT (2-layer, 8-head) Trainium2 kernel, 8-core SPMD — sort-based layer 1.

Layer 1 (head-parallel, one head per core) avoids materializing the [N,N]
score matrix entirely.  With g = f1_i + f2_j and p = exp(leakyrelu(g))*m:
    p = A[i]*q2'[j]*m          when g < 0   (A = e^{0.2 f1},  q2' = e^{0.2 f2})
    p = B[i]*u2'[j]*m          when g >= 0  (B = e^{f1},      u2' = e^{f2})
Sorting j by f2 and i by the cutoff c(i) = #{j : f2_j < -f1_i} makes the
branch a block predicate: for column-block k (128 sorted i's) and row-chunk
q (128 sorted j's), q < b_lo(k) is pure branch-1, q > b_hi(k) pure branch-2,
and the few boundary chunks are host-presplit into bd1/bd2 = m*1[branch].
The PE then consumes the raw permuted mask (fp8, exact for 0/1) as the
stationary operand and streams [Wh*q2'|q2'] / [Wh*u2'|u2'] (66 bf16 rows)
as the moving operand, accumulating S1/S2 per block in PSUM; the drain
combines num = A*S1 + B*S2 on Act/DVE.  No elementwise pass ever touches
an [N,N] tensor, so the phase is paced by the 16.7MB fp8 mask DMA.

Layer 2 (row-parallel, 512 rows per core) keeps the dense-score pipeline but
with scores p~ = max(q2o[j], r8[i]*u2o[j]) * m  (r8 = e^{0.8 f1o}; the
e^{0.2 f1_i} factor cancels in the softmax): one fused 2-scalar tensor_scalar
(DVE 4x mode) plus one mask multiply split DVE/Pool, then the attention
matmul with plain [Wh2|1] as the moving operand.  The raw PSUM accumulators
ship back and the host finishes normalize/elu/log_softmax.

The host does the O(N*F) prep (projections, exp vectors, sorts, mask
permutations and boundary splits, inter-layer elu/concat) in numpy; both
bass programs are built at first kernel() call from the inputs' cutoff
structure (uniform across cores so the programs stay SPMD).
"""

import sys

for p in ("/opt/trn_rl_repo", "/opt/pypackages"):
    if p not in sys.path:
        sys.path.append(p)

import numpy as np
import ml_dtypes

import concourse.bass as bass
import concourse.bacc as bacc
import concourse.tile as tile
from concourse import mybir
from concourse.bass_utils import run_bass_kernel_spmd

BF16 = mybir.dt.bfloat16
FP8 = mybir.dt.float8e4
F32 = mybir.dt.float32
OP = mybir.AluOpType
AF = mybir.ActivationFunctionType

NPBF = ml_dtypes.bfloat16
NPF8 = ml_dtypes.float8_e4m3fn

N, FIN, HID, HEADS, FOUT = 4096, 512, 64, 8, 256
NCORES = 8
NCH = N // 128          # 32 row chunks / column blocks
WC1 = HID + 2           # 64 features | den | pad
WC2 = FOUT + 1          # 256 features | den
ROWS2 = N // NCORES     # 512 output rows per core in layer 2
ALPHA = 0.2


# --------------------------------------------------------------------------
# phase 1: sort-based head attention.  spans = ((b_lo, b_hi), ...) per block,
# uniform across heads; bdoff[k] = offset of block k's boundary tiles.
# --------------------------------------------------------------------------
def build_phase1(spans):
    nc = bacc.Bacc("TRN2", target_bir_lowering=False, debug=False,
                   enable_asserts=False)
    sb = sum(hi - lo + 1 for lo, hi in spans)
    bdoff = []
    off = 0
    for lo, hi in spans:
        bdoff.append(off)
        off += hi - lo + 1

    # slab r holds mask columns [256r, 256r+256) for all 32 row chunks,
    # host-packed contiguous per partition: [p][q][c] with c in-block col.
    # Boundary-chunk blocks of the slab are pre-split by the host to the
    # branch-2 part (bd2); the branch-1 part ships separately as bd1.
    mps = nc.dram_tensor("mps", [16, 128, NCH * 256], FP8,
                         kind="ExternalInput")
    bd1 = nc.dram_tensor("bd1", [128, sb * 128], FP8, kind="ExternalInput")
    rqd = nc.dram_tensor("rqd", [128, NCH * WC1], BF16, kind="ExternalInput")
    rud = nc.dram_tensor("rud", [128, NCH * WC1], BF16, kind="ExternalInput")
    abd = nc.dram_tensor("abd", [128, NCH * 2], F32, kind="ExternalInput")
    numo = nc.dram_tensor("numo", [128, NCH * WC1], BF16,
                          kind="ExternalOutput")

    with tile.TileContext(nc) as tc:
        with tc.tile_pool(name="consts", bufs=1) as consts:
            rq = consts.tile([128, NCH * WC1], BF16)
            ru = consts.tile([128, NCH * WC1], BF16)
            ab = consts.tile([128, NCH * 2], F32)
            bd1s = consts.tile([128, sb * 128], FP8)
            stage = consts.tile([128, NCH * WC1], BF16)
            h1 = 8 * WC1
            nc.gpsimd.dma_start(out=rq[:, 0:h1], in_=rqd[:, 0:h1])
            nc.gpsimd.dma_start(out=ru[:, 0:h1], in_=rud[:, 0:h1])
            nc.gpsimd.dma_start(out=rq[:, h1:], in_=rqd[:, h1:])
            nc.gpsimd.dma_start(out=ru[:, h1:], in_=rud[:, h1:])
            nc.gpsimd.dma_start(out=ab[:], in_=abd[:, :])
            bq = (sb + 3) // 4 * 128
            for i in range(4):
                c0, c1 = i * bq, min((i + 1) * bq, sb * 128)
                if c0 < c1:
                    nc.scalar.dma_start(out=bd1s[:, c0:c1],
                                        in_=bd1[:, c0:c1])
            with (
                tc.tile_pool(name="slabs", bufs=4) as slabs,
                tc.tile_pool(name="ps1", bufs=4, space="PSUM") as ps1p,
                tc.tile_pool(name="ps2", bufs=4, space="PSUM") as ps2p,
                tc.tile_pool(name="ep", bufs=4) as ep,
            ):
                for r in range(16):          # 2 column blocks per round
                    slab = slabs.tile([128, NCH * 256], FP8, name="slab",
                                      tag="slab")
                    eng = nc.sync if r % 2 == 0 else nc.gpsimd
                    hw = NCH * 128
                    for hf in range(2):
                        eng.dma_start(
                            out=slab[:, hf * hw:(hf + 1) * hw],
                            in_=bass.AP(
                                tensor=mps,
                                offset=r * 128 * NCH * 256 + hf * hw,
                                ap=[[NCH * 256, 128], [1, hw]]))
                    ks = (2 * r, 2 * r + 1)
                    ps1 = {k: ps1p.tile([128, WC1], F32, name=f"ps1_{k}",
                                        tag="ps1") for k in ks}
                    ps2 = {k: ps2p.tile([128, WC1], F32, name=f"ps2_{k}",
                                        tag="ps2") for k in ks}
                    for q in range(NCH):
                        for k in ks:
                            lo, hi = spans[k]
                            col = q * 256 + (k % 2) * 128
                            rqs = rq[:, q * WC1:(q + 1) * WC1]
                            rus = ru[:, q * WC1:(q + 1) * WC1]
                            if q < lo:
                                nc.tensor.matmul(
                                    out=ps1[k][:],
                                    lhsT=slab[:, col:col + 128],
                                    rhs=rqs, start=(q == 0), stop=False)
                            elif q > hi:
                                nc.tensor.matmul(
                                    out=ps2[k][:],
                                    lhsT=slab[:, col:col + 128],
                                    rhs=rus, start=False, stop=(q == NCH - 1))
                            else:
                                i0 = (bdoff[k] + q - lo) * 128
                                nc.tensor.matmul(
                                    out=ps1[k][:],
                                    lhsT=bd1s[:, i0:i0 + 128],
                                    rhs=rqs, start=(q == 0), stop=(q == hi))
                                nc.tensor.matmul(
                                    out=ps2[k][:],
                                    lhsT=slab[:, col:col + 128],
                                    rhs=rus, start=(q == lo),
                                    stop=(q == NCH - 1))
                    for k in ks:
                        t1 = ep.tile([128, WC1], BF16, name="t1", tag="t1")
                        nc.scalar.activation(out=t1[:], in_=ps1[k][:],
                                             func=AF.Copy,
                                             scale=ab[:, 2 * k:2 * k + 1])
                        nc.vector.scalar_tensor_tensor(
                            out=stage[:, k * WC1:(k + 1) * WC1],
                            in0=ps2[k][:], scalar=ab[:, 2 * k + 1:2 * k + 2],
                            in1=t1[:], op0=OP.mult, op1=OP.add)
                    c0, c1 = 2 * r * WC1, (2 * r + 2) * WC1
                    nc.scalar.dma_start(out=numo[:, c0:c1],
                                        in_=stage[:, c0:c1])

    nc.compile()
    return nc


# --------------------------------------------------------------------------
# phase 2: dense-score layer-2 attention for 512 rows per core.
# --------------------------------------------------------------------------
def build_phase2():
    nc = bacc.Bacc("TRN2", target_bir_lowering=False, debug=False,
                   enable_asserts=False)
    rows = ROWS2
    rch = rows // 128
    XS = 368             # end of the DVE tensor-mul range; rest on Pool

    wh2 = nc.dram_tensor("wh2", [128, NCH * WC2], BF16, kind="ExternalInput")
    m2 = nc.dram_tensor("m2", [128, NCH * rows], BF16, kind="ExternalInput")
    r8d = nc.dram_tensor("r8d", [1, rows], BF16, kind="ExternalInput")
    qud = nc.dram_tensor("qud", [128, NCH * 3], F32, kind="ExternalInput")
    out = nc.dram_tensor("out", [128, rch * WC2], F32, kind="ExternalOutput")

    with tile.TileContext(nc) as tc:
        with tc.tile_pool(name="consts", bufs=1) as consts:
            qu = consts.tile([128, NCH * 3], F32)
            nc.sync.dma_start(out=qu[:], in_=qud[:, :])
            r8b = consts.tile([128, rows], BF16)
            nc.sync.dma_start(
                out=r8b[:],
                in_=bass.AP(tensor=r8d, offset=0, ap=[[0, 128], [1, rows]]))
            wh2sb = consts.tile([128, NCH * WC2], BF16)
            m2sb = consts.tile([128, NCH * rows], BF16)
            for i in range(16):
                nc.scalar.dma_start(
                    out=m2sb[:, i * 2 * rows:(i + 1) * 2 * rows],
                    in_=m2[:, i * 2 * rows:(i + 1) * 2 * rows])
                if i < 8:
                    nc.sync.dma_start(
                        out=wh2sb[:, i * 4 * WC2:(i + 1) * 4 * WC2],
                        in_=wh2[:, i * 4 * WC2:(i + 1) * 4 * WC2])

            with (
                tc.tile_pool(name="t2pool", bufs=6) as t2pool,
                tc.tile_pool(name="t3pool", bufs=6) as t3pool,
                tc.tile_pool(name="ps4", bufs=rch, space="PSUM") as ps4,
            ):
                po = [ps4.tile([128, WC2], F32, name=f"po{_i}", tag="po")
                      for _i in range(rch)]
                for jc in range(NCH):
                    ms = m2sb[:, jc * rows:(jc + 1) * rows]
                    t2 = t2pool.tile([128, rows], BF16)
                    nc.vector.tensor_scalar(
                        out=t2[:], in0=r8b[:],
                        scalar1=qu[:, 3 * jc:3 * jc + 1],
                        scalar2=qu[:, 3 * jc + 1:3 * jc + 2],
                        op0=OP.mult, op1=OP.max)
                    t3 = t3pool.tile([128, rows], BF16)
                    nc.vector.tensor_mul(t3[:, 0:XS], t2[:, 0:XS],
                                         ms[:, 0:XS])
                    nc.gpsimd.tensor_mul(t3[:, XS:rows], t2[:, XS:],
                                         ms[:, XS:rows])
                    for ic in range(rch):
                        nc.tensor.matmul(
                            out=po[ic][:],
                            lhsT=t3[:, ic * 128:(ic + 1) * 128],
                            rhs=wh2sb[:, jc * WC2:(jc + 1) * WC2],
                            start=(jc == 0), stop=(jc == NCH - 1))

                # raw accumulators out; normalize/elu/log_softmax on host
                pod = consts.tile([128, rch * WC2], F32)
                for ic in range(rch):
                    osl = slice(ic * WC2, (ic + 1) * WC2)
                    if ic % 2 == 0:
                        nc.vector.tensor_copy(out=pod[:, osl], in_=po[ic][:])
                    else:
                        nc.scalar.activation(out=pod[:, osl], in_=po[ic][:],
                                             func=AF.Copy)
                    deng = (nc.sync, nc.scalar, nc.sync, nc.scalar)[ic]
                    deng.dma_start(out=out[:, osl], in_=pod[:, osl])

    nc.compile()
    return nc


_CACHE = {}


def _get_programs():
    return _CACHE["p1"], _CACHE["p2"]


# --------------------------------------------------------------------------
# host-side prep
# --------------------------------------------------------------------------
def _sort_structure(f1, f2):
    """sigma (rows by f2), tau (cols by cutoff), cutoffs c, block bounds."""
    sigma = np.argsort(f2, kind="stable")
    f2s = f2[sigma]
    c = np.searchsorted(f2s, -f1, side="left")   # branch-1 count per col
    tau = np.argsort(c, kind="stable")
    cs = c[tau]
    b = cs // 128                                 # boundary chunk per col
    lo = np.minimum(b.reshape(NCH, 128).min(axis=1), NCH - 1)
    hi = np.minimum(b.reshape(NCH, 128).max(axis=1), NCH - 1)
    return sigma, tau, cs, lo, hi


def prep_phase1(x, adj, W_heads, a1_heads, a2_heads):
    maskT8 = (adj > 0).T.astype(NPF8)            # maskT[j, i] = adj[i, j]
    mu8 = maskT8.view(np.uint8)

    heads = []
    los = np.full(NCH, NCH - 1, np.int64)
    his = np.zeros(NCH, np.int64)
    for h in range(HEADS):
        Wh = (x @ W_heads[h]).astype(np.float32)          # [N, 64]
        f1 = Wh @ a1_heads[h]
        f2 = Wh @ a2_heads[h]
        sigma, tau, cs, lo, hi = _sort_structure(f1, f2)
        los = np.minimum(los, lo)
        his = np.maximum(his, hi)
        heads.append((Wh, f1, f2, sigma, tau, cs))
    spans = tuple((int(l), int(h)) for l, h in zip(los, his))
    sb = sum(h - l + 1 for l, h in spans)

    in1 = []
    for h in range(HEADS):
        Wh, f1, f2, sigma, tau, cs = heads[h]
        mp = mu8[np.ix_(sigma, tau)].copy()               # [N, N] permuted
        # boundary splits: bd1 ships separately; the branch-2 half
        # overwrites the boundary blocks of mp (consumed via the slab)
        bd1 = np.zeros((128, sb * 128), np.uint8)
        idx = 0
        for k, (lo, hi) in enumerate(spans):
            ck = cs[k * 128:(k + 1) * 128]                # cutoffs, this block
            for q in range(lo, hi + 1):
                mblk = mp[q * 128:(q + 1) * 128, k * 128:(k + 1) * 128]
                r = np.arange(q * 128, q * 128 + 128)[:, None]
                br1 = (r < ck[None, :])
                bd1[:, idx * 128:(idx + 1) * 128] = np.where(br1, mblk, 0)
                mp[q * 128:(q + 1) * 128,
                   k * 128:(k + 1) * 128] = np.where(br1, 0, mblk)
                idx += 1
        # slabs: [16][p][q][256] = mp[128q + p, 256r + c]
        mps = np.ascontiguousarray(
            mp.reshape(NCH, 128, 16, 256).transpose(2, 1, 0, 3)
            .reshape(16, 128, NCH * 256)).view(NPF8)
        f2s = f2[sigma]
        q2p = np.exp(ALPHA * f2s).astype(np.float32)      # e^{0.2 f2}
        u2p = np.exp(f2s).astype(np.float32)              # e^{f2}
        Whs = Wh[sigma]                                   # sorted rows
        rqf = np.concatenate([Whs * q2p[:, None], q2p[:, None],
                              np.zeros((N, 1), np.float32)], axis=1)
        ruf = np.concatenate([Whs * u2p[:, None], u2p[:, None],
                              np.zeros((N, 1), np.float32)], axis=1)
        # [p][q][f] layout
        rqd = np.ascontiguousarray(
            rqf.reshape(NCH, 128, WC1).transpose(1, 0, 2)
            .reshape(128, NCH * WC1)).astype(NPBF)
        rud = np.ascontiguousarray(
            ruf.reshape(NCH, 128, WC1).transpose(1, 0, 2)
            .reshape(128, NCH * WC1)).astype(NPBF)
        f1t = f1[tau]
        abf = np.stack([np.exp(ALPHA * f1t), np.exp(f1t)],
                       axis=1).astype(np.float32)         # [N, 2] A|B
        abd = np.ascontiguousarray(
            abf.reshape(NCH, 128, 2).transpose(1, 0, 2)
            .reshape(128, NCH * 2))
        in1.append({"mps": mps, "bd1": bd1.view(NPF8),
                    "rqd": rqd, "rud": rud, "abd": abd})
    return in1, heads, spans


def finish_phase1(r1, heads):
    """num/den -> h (elu'd, concatenated) in natural node order."""
    H = np.zeros((N, HEADS * HID), np.float32)
    for h in range(HEADS):
        tau = heads[h][4]
        numo = r1[h]["numo"].astype(np.float32)           # [128, NCH*WC1]
        ns = numo.reshape(128, NCH, WC1).transpose(1, 0, 2).reshape(N, WC1)
        hv = ns[:, 0:HID] / ns[:, HID:HID + 1]            # sorted cols
        hn = np.empty_like(hv)
        hn[tau] = hv                                      # un-permute
        H[:, h * HID:(h + 1) * HID] = np.where(hn > 0, hn, np.expm1(hn))
    return H


def prep_phase2(H, adj, W_out, a1_out, a2_out):
    maskT8 = (adj > 0).T.astype(NPBF)
    Wh2 = (H @ W_out).astype(np.float32)                  # [N, 256]
    f1o = Wh2 @ a1_out
    f2o = Wh2 @ a2_out
    wh2f = np.concatenate([Wh2, np.ones((N, 1), np.float32)], axis=1)
    wh2d = np.ascontiguousarray(
        wh2f.reshape(NCH, 128, WC2).transpose(1, 0, 2)
        .reshape(128, NCH * WC2)).astype(NPBF)
    q2o = np.exp(ALPHA * f2o)
    quf = np.stack([np.exp(f2o), q2o, -q2o],
                   axis=1).astype(np.float32)             # u2o | q2o | -q2o
    qud = np.ascontiguousarray(
        quf.reshape(NCH, 128, 3).transpose(1, 0, 2).reshape(128, NCH * 3))
    in2 = []
    for c in range(NCORES):
        rs = slice(c * ROWS2, (c + 1) * ROWS2)
        m2 = np.ascontiguousarray(
            maskT8.reshape(NCH, 128, N)[:, :, rs]
            .transpose(1, 0, 2).reshape(128, NCH * ROWS2))
        r8 = np.exp(0.8 * f1o[rs]).astype(NPBF)[None, :]
        in2.append({"wh2": wh2d, "m2": m2, "r8d": r8, "qud": qud})
    return in2


def kernel(x, adj, W_heads, a1_heads, a2_heads, W_out, a1_out, a2_out, **_):
    x = np.asarray(x, dtype=np.float32)
    adj = np.asarray(adj)
    W_heads = np.asarray(W_heads, dtype=np.float32)
    a1_heads = np.asarray(a1_heads, dtype=np.float32)
    a2_heads = np.asarray(a2_heads, dtype=np.float32)
    W_out = np.asarray(W_out, dtype=np.float32)
    a1_out = np.asarray(a1_out, dtype=np.float32)
    a2_out = np.asarray(a2_out, dtype=np.float32)

    in1, heads, spans = prep_phase1(x, adj, W_heads, a1_heads, a2_heads)
    if _CACHE.get("spans") != spans:
        _CACHE["p1"] = build_phase1(spans)
        _CACHE["spans"] = spans
    if "p2" not in _CACHE:
        _CACHE["p2"] = build_phase2()
    p1, p2 = _CACHE["p1"], _CACHE["p2"]

    r1 = run_bass_kernel_spmd(p1, in1, core_ids=list(range(NCORES))).results
    H = finish_phase1(r1, heads)
    in2 = prep_phase2(H, adj, W_out, a1_out, a2_out)
    r2 = run_bass_kernel_spmd(p2, in2, core_ids=list(range(NCORES))).results
    # host epilogue: normalize, elu, log_softmax per core's raw accumulators
    outs = []
    for c in range(NCORES):
        po = np.asarray(r2[c]["out"], np.float32)         # [128, rch*WC2]
        po = po.reshape(128, ROWS2 // 128, WC2).transpose(1, 0, 2) \
               .reshape(ROWS2, WC2)
        an = po[:, 0:FOUT] / po[:, FOUT:FOUT + 1]
        el = np.where(an > 0, an, np.expm1(an))
        el -= np.log(np.exp(el).sum(axis=1, keepdims=True))
        outs.append(el)
    return np.concatenate(outs, axis=0).astype(np.float32)
